# revision 24
# baseline (speedup 1.0000x reference)
"""Trainium2 Bass kernel for a Swin-style transformer block
(windowed attention with RoPE + SwiGLU MLP with sub-LN).

Sharding: data-parallel over batch B=8 -> one image per NeuronCore.
Each core computes the full block for its image in window-partitioned,
feature-major layout; the host does window (un)partitioning, LN-affine
folding into the projection weights, and RoPE table generation.

v2 design notes (vs the earlier gpsimd/DVE-heavy version):
- LN statistics via PE ones-matmuls into [1, pc] PSUM rows; partition
  broadcasts via K=1 ones-matmuls (no gpsimd partition_all_reduce).
- rstd = Exp(-0.5*Ln(var+eps)) on ScalarE so the activation table stays
  in natural_log_exp_and_others (shared with attention's Exp); only the
  MLP's Silu forces a table switch.
- Softmax denominators via reciprocal_approx_fast (5x faster than the
  iterative reciprocal).
- bf16 residual stream; all bulk DVE elementwise ops in bf16 (2x mode);
  ScalarE does the PSUM->SBUF evacuations.
- x loaded from fp32 DRAM as bf16 by DMAing the high 2 bytes
  (truncation); host emulation shows the extra error is ~1e-4.
"""
import numpy as np
import ml_dtypes
from contextlib import ExitStack

import concourse.bass as bass
import concourse.tile as tile
from concourse import bacc, mybir
from concourse.bass_utils import run_bass_kernel_spmd

BF16NP = ml_dtypes.bfloat16
F32 = mybir.dt.float32
BF16 = mybir.dt.bfloat16
OP = mybir.AluOpType
AF = mybir.ActivationFunctionType
AX = mybir.AxisListType

DIM = 768
HEADS = 12
HD = 64
HID = 2048
EPS = 1e-6
WS = 14
NTOK = WS * WS          # 196 tokens per window
B, H, W = 8, 64, 64
NWIN = 25               # 5x5 windows per image
TOKS = NWIN * NTOK      # 4900
KT = DIM // 128         # 6 feature tiles
MT = HID // 128         # 16 hid tiles
N_CORES = 8
P = 128
PC2 = 2 * NTOK          # 392: max columns per window-pair

_cache = {}
SILU_MODE = "silu"   # "silu" (HW) or "sigmoid" (CoreSim lacks Silu)
RECIP_MODE = "lnexp"   # "lnexp" (ScalarE Exp(-Ln(z))), "approx", or "exact"
X_LOAD = "trunc"        # "trunc" (bf16 hi-bytes DMA) or "f32"


def _rope_tables():
    dim, pt, theta = 32, 16.0, 10000.0
    freqs = 1.0 / theta ** (np.arange(0, dim, 2, dtype=np.float32) / dim)
    f1 = np.repeat((np.arange(WS, dtype=np.float32) / WS * pt)[:, None] * freqs[None, :], 2, axis=-1)
    f = np.concatenate([
        np.broadcast_to(f1[:, None, :], (WS, WS, dim)),
        np.broadcast_to(f1[None, :, :], (WS, WS, dim)),
    ], -1).reshape(NTOK, 2 * dim)
    return np.cos(f), np.sin(f)   # [196, 64] fp32


def _emit(nc, tc, ctx, aps, has_b, nwin_total=NWIN, loop_n=1):
    pairs = []
    w = 0
    while w < nwin_total:
        pairs.append((w, w + 1) if w + 1 < nwin_total else (w,))
        w += 2

    # x DRAM view: [768, toks, 2] bf16 (fp32 reinterpreted); [..., 1] is
    # the high half = truncated bf16.
    if X_LOAD == "trunc":
        xTb = aps["xT"][:, :, 1:2].rearrange("(k p) n one -> p k (n one)", p=P)
    else:
        xT32 = aps["xT"].rearrange("(k p) n -> p k n", p=P)
    yT = aps["yT"].rearrange("(k p) n -> p k n", p=P)
    w3d = aps["w3"].rearrange("(k p) m -> p k m", p=P)    # [128, 16, 768]

    consts = ctx.enter_context(tc.tile_pool(name="consts", bufs=1))
    wpool = ctx.enter_context(tc.tile_pool(name="weights", bufs=1))
    w3pool = ctx.enter_context(tc.tile_pool(name="w3s", bufs=2))
    xpool = ctx.enter_context(tc.tile_pool(name="x", bufs=2))
    sqpool = ctx.enter_context(tc.tile_pool(name="sq", bufs=2))
    rowpool = ctx.enter_context(tc.tile_pool(name="rows", bufs=1))
    bcpool = ctx.enter_context(tc.tile_pool(name="bc", bufs=2))
    hpool = ctx.enter_context(tc.tile_pool(name="h", bufs=1))
    tpool = ctx.enter_context(tc.tile_pool(name="tmp", bufs=2))
    qkpool = ctx.enter_context(tc.tile_pool(name="qk", bufs=2))
    ropepool = ctx.enter_context(tc.tile_pool(name="rope", bufs=2))
    vpool = ctx.enter_context(tc.tile_pool(name="v", bufs=2))
    epool = ctx.enter_context(tc.tile_pool(name="exp", bufs=2))
    zpool = ctx.enter_context(tc.tile_pool(name="z", bufs=2))
    opool = ctx.enter_context(tc.tile_pool(name="ohat", bufs=1))
    x1pool = ctx.enter_context(tc.tile_pool(name="x1", bufs=2))
    mlppool = ctx.enter_context(tc.tile_pool(name="mlp", bufs=2))
    gpool = ctx.enter_context(tc.tile_pool(name="g", bufs=1))
    ypool = ctx.enter_context(tc.tile_pool(name="y", bufs=2))

    ps_mm = ctx.enter_context(tc.tile_pool(name="psmm", bufs=2, space="PSUM"))
    ps_att = ctx.enter_context(tc.tile_pool(name="psatt", bufs=3, space="PSUM"))
    ps_rot = ctx.enter_context(tc.tile_pool(name="psrot", bufs=1, space="PSUM"))
    ps_stat = ctx.enter_context(tc.tile_pool(name="psstat", bufs=1, space="PSUM"))

    # --- constants / weights in SBUF ---
    def load_w(name, kdim, mdim):
        t = wpool.tile([P, kdim // P, mdim], BF16, tag=name)
        nc.sync.dma_start(t[:], aps[name].rearrange("(k p) m -> p k m", p=P))
        return t

    wq = load_w("wq", DIM, DIM)
    wk = load_w("wk", DIM, DIM)
    wv = load_w("wv", DIM, DIM)
    wp = load_w("wp", DIM, DIM)
    w1 = load_w("w1", DIM, HID)
    w2 = load_w("w2", DIM, HID)

    cos2 = consts.tile([P, PC2], BF16, tag="cos2")
    nc.sync.dma_start(cos2[:], aps["cos2"][:])
    sin2 = consts.tile([P, PC2], BF16, tag="sin2")
    nc.sync.dma_start(sin2[:], aps["sin2"][:])
    r2t = consts.tile([P, P], BF16, tag="r2t")
    nc.sync.dma_start(r2t[:], aps["r2t"][:])
    w3c = consts.tile([1, DIM], BF16, tag="w3c")
    nc.sync.dma_start(w3c[:], aps["w3c"][:])
    colones = consts.tile([P, 1], BF16, tag="colones")
    nc.vector.memset(colones[:], 1.0)
    rowones = consts.tile([1, P], BF16, tag="rowones")
    nc.vector.memset(rowones[:], 1.0)
    epsc = consts.tile([1, 1], F32, tag="epsc")
    nc.vector.memset(epsc[:], EPS)

    def bias_col(name, feat):
        if aps.get(name) is None:
            return None
        t = consts.tile([P, feat // P], F32, tag=name)
        nc.sync.dma_start(t[:], aps[name].rearrange("(k p) -> p k", p=P))
        return t

    qb = bias_col("qb", DIM)
    kb = bias_col("kb", DIM)
    vb = bias_col("vb", DIM)
    pb = bias_col("pb", DIM)
    w1b = bias_col("w1b", HID)
    w2b = bias_col("w2b", HID)
    w3b = bias_col("w3b", DIM)
    vbr = None
    if has_b.get("vb"):
        vbr = consts.tile([1, DIM], BF16, tag="vbr")
        nc.sync.dma_start(vbr[:], aps["vbr"][:])

    CHUNKS = [(0, P), (P, NTOK - P)]   # [128, 68] token chunks per window

    def emit_all_pairs():
        for wins in pairs:
            nwin = len(wins)
            pc = NTOK * nwin
            c0 = wins[0] * NTOK

            xb = xpool.tile([P, KT, PC2], BF16, tag="xb")
            if X_LOAD == "trunc":
                for k in range(KT):
                    nc.sync.dma_start(xb[:, k, :pc], xTb[:, k, c0:c0 + pc])
            else:
                x32 = xpool.tile([P, KT, PC2], F32, tag="x32")
                nc.sync.dma_start(x32[:, :, :pc], xT32[:, :, c0:c0 + pc])
                for k in range(KT):
                    nc.scalar.activation(out=xb[:, k, :pc], in_=x32[:, k, :pc],
                                         func=AF.Copy, bias=0.0, scale=1.0)

            # ---------- LN stats: sums via PE, tail via ScalarE+DVE ----
            def ln_stats(src, kt, inv_n):
                s1 = ps_stat.tile([1, PC2], F32, tag="s1")
                s2 = ps_stat.tile([1, PC2], F32, tag="s2")
                for k in range(kt):
                    xsq = sqpool.tile([P, PC2], BF16, tag="xsq")
                    nc.vector.tensor_tensor(out=xsq[:, :pc], in0=src[:, k, :pc],
                                            in1=src[:, k, :pc], op=OP.mult)
                    nc.tensor.matmul(s1[:, :pc], lhsT=colones[:, 0:1],
                                     rhs=src[:, k, :pc], start=(k == 0),
                                     stop=(k == kt - 1), skip_group_check=True)
                    nc.tensor.matmul(s2[:, :pc], lhsT=colones[:, 0:1],
                                     rhs=xsq[:, :pc], start=(k == 0),
                                     stop=(k == kt - 1), skip_group_check=True)
                return s1, s2

            def ln_tail(s1, s2, inv_n, want_bcast_mu=True):
                # mu = s1/n (bf16 row); var = s2/n - mu^2 (fp32 exact scale)
                mu_row = rowpool.tile([1, PC2], BF16, tag="mur")
                nc.scalar.activation(out=mu_row[:, :pc], in_=s1[:, :pc],
                                     func=AF.Identity, bias=0.0, scale=inv_n)
                msq = rowpool.tile([1, PC2], F32, tag="msq")
                nc.scalar.activation(out=msq[:, :pc], in_=s1[:, :pc],
                                     func=AF.Square, bias=0.0, scale=inv_n)
                varr = rowpool.tile([1, PC2], F32, tag="varr")
                nc.vector.scalar_tensor_tensor(out=varr[:, :pc], in0=s2[:, :pc],
                                               scalar=inv_n, in1=msq[:, :pc],
                                               op0=OP.mult, op1=OP.subtract)
                lnv = rowpool.tile([1, PC2], F32, tag="lnv")
                nc.scalar.activation(out=lnv[:, :pc], in_=varr[:, :pc],
                                     func=AF.Ln, bias=epsc[:], scale=1.0)
                rstd_row = rowpool.tile([1, PC2], BF16, tag="rstdr")
                nc.scalar.activation(out=rstd_row[:, :pc], in_=lnv[:, :pc],
                                     func=AF.Exp, bias=0.0, scale=-0.5)
                rsb_ps = ps_rot.tile([P, PC2], F32, tag="rot")
                nc.tensor.matmul(rsb_ps[:, :pc], lhsT=rowones[:, 0:P],
                                 rhs=rstd_row[:, :pc], start=True, stop=True)
                rstd_b = bcpool.tile([P, PC2], BF16, tag="rstdb")
                nc.scalar.activation(out=rstd_b[:, :pc], in_=rsb_ps[:, :pc],
                                     func=AF.Copy, bias=0.0, scale=1.0)
                mu_b = None
                if want_bcast_mu:
                    mub_ps = ps_rot.tile([P, PC2], F32, tag="rot")
                    nc.tensor.matmul(mub_ps[:, :pc], lhsT=rowones[:, 0:P],
                                     rhs=mu_row[:, :pc], start=True, stop=True)
                    mu_b = bcpool.tile([P, PC2], BF16, tag="mub")
                    nc.scalar.activation(out=mu_b[:, :pc], in_=mub_ps[:, :pc],
                                         func=AF.Copy, bias=0.0, scale=1.0)
                return mu_row, mu_b, rstd_b

            def ln_apply(src, mu_b, rstd_b, tag):
                hh = hpool.tile([P, KT, PC2], BF16, tag=tag)
                for k in range(KT):
                    tmpc = tpool.tile([P, PC2], BF16, tag="tmpc")
                    nc.vector.tensor_tensor(out=tmpc[:, :pc], in0=src[:, k, :pc],
                                            in1=mu_b[:, :pc], op=OP.subtract)
                    nc.vector.tensor_tensor(out=hh[:, k, :pc], in0=tmpc[:, :pc],
                                            in1=rstd_b[:, :pc], op=OP.mult)
                return hh

            s1a, s2a = ln_stats(xb, KT, 1.0 / DIM)
            _, mu1b, rstd1b = ln_tail(s1a, s2a, 1.0 / DIM)
            h1 = ln_apply(xb, mu1b, rstd1b, "h1")

            # ---------- QKV + RoPE (feature-major q/k) ----------
            def emit_qk(wmat, bcol, dest):
                for m in range(KT):
                    ps = ps_mm.tile([P, PC2], F32, tag="mm")
                    for k in range(KT):
                        nc.tensor.matmul(ps[:, :pc], lhsT=wmat[:, k, m * P:(m + 1) * P],
                                         rhs=h1[:, k, :pc], start=(k == 0), stop=(k == KT - 1))
                    qs = ropepool.tile([P, PC2], BF16, tag="qs")
                    if bcol is None:
                        nc.scalar.activation(out=qs[:, :pc], in_=ps[:, :pc],
                                             func=AF.Copy, bias=0.0, scale=1.0)
                    else:
                        nc.scalar.activation(out=qs[:, :pc], in_=ps[:, :pc],
                                             func=AF.Identity, bias=bcol[:, m:m + 1],
                                             scale=1.0)
                    rot = ps_rot.tile([P, PC2], F32, tag="rot")
                    nc.tensor.matmul(rot[:, :pc], lhsT=r2t[:], rhs=qs[:, :pc],
                                     start=True, stop=True)
                    t1 = ropepool.tile([P, PC2], BF16, tag="t1")
                    nc.vector.tensor_tensor(out=t1[:, :pc], in0=qs[:, :pc],
                                            in1=cos2[:, :pc], op=OP.mult)
                    t2 = ropepool.tile([P, PC2], BF16, tag="t2")
                    nc.vector.tensor_tensor(out=t2[:, :pc], in0=rot[:, :pc],
                                            in1=sin2[:, :pc], op=OP.mult)
                    nc.vector.tensor_tensor(out=dest[:, m, :pc], in0=t1[:, :pc],
                                            in1=t2[:, :pc], op=OP.add)

            qhat = qkpool.tile([P, KT, PC2], BF16, tag="qhat")
            khat = qkpool.tile([P, KT, PC2], BF16, tag="khat")
            emit_qk(wq, qb, qhat)
            emit_qk(wk, kb, khat)

            # ---------- V (token-major, per window, with ones column) --
            v_ts = []
            for wi in range(nwin):
                wcol = wi * NTOK
                vt = []
                for ci, (cs, cn) in enumerate(CHUNKS):
                    v_t = vpool.tile([P, HEADS, HD + 1], BF16, tag=f"v{ci}")
                    nc.vector.memset(v_t[:, :, HD:HD + 1], 1.0)
                    for half in range(2):
                        nh = DIM // 2
                        vps = ps_mm.tile([P, PC2], F32, tag="mm")
                        for k in range(KT):
                            nc.tensor.matmul(vps[0:cn, 0:nh],
                                             lhsT=h1[:, k, wcol + cs:wcol + cs + cn],
                                             rhs=wv[:, k, half * nh:(half + 1) * nh],
                                             start=(k == 0), stop=False if vbr is not None else (k == KT - 1),
                                             skip_group_check=True)
                        if vbr is not None:
                            nc.tensor.matmul(vps[0:cn, 0:nh],
                                             lhsT=rowones[:, 0:cn],
                                             rhs=vbr[:, half * nh:(half + 1) * nh],
                                             start=False, stop=True, skip_group_check=True)
                        nc.scalar.activation(
                            out=v_t[0:cn, half * (HEADS // 2):(half + 1) * (HEADS // 2), 0:HD],
                            in_=vps[0:cn, 0:nh].rearrange("p (h d) -> p h d", d=HD),
                            func=AF.Copy, bias=0.0, scale=1.0)
                    vt.append(v_t)
                v_ts.append(vt)

            # ---------- attention per window/head ----------
            ohat = opool.tile([P, KT, PC2], BF16, tag="ohat")
            for wi in range(nwin):
                wcol = wi * NTOK
                for hh in range(HEADS):
                    r0 = 64 * (hh % 2)
                    g6 = hh // 2
                    qsl = qhat[r0:r0 + 64, g6, wcol:wcol + NTOK]
                    ksl = khat[r0:r0 + 64, g6, wcol:wcol + NTOK]
                    es = []
                    for ci, (cs, cn) in enumerate(CHUNKS):
                        sps = ps_att.tile([P, PC2], F32, tag="att")
                        nc.tensor.matmul(sps[0:cn, 0:NTOK], lhsT=ksl[:, cs:cs + cn],
                                         rhs=qsl, start=True, stop=True)
                        e = epool.tile([P, NTOK], BF16, tag=f"e{ci}")
                        nc.scalar.activation(out=e[0:cn, :], in_=sps[0:cn, 0:NTOK],
                                             func=AF.Exp, bias=0.0, scale=1.0)
                        es.append(e)
                    ops = ps_att.tile([P, PC2], F32, tag="att")
                    for ci, (cs, cn) in enumerate(CHUNKS):
                        nc.tensor.matmul(ops[0:HD + 1, 0:NTOK], lhsT=v_ts[wi][ci][0:cn, hh, :],
                                         rhs=es[ci][0:cn, :], start=(ci == 0), stop=(ci == 1))
                    zrow = zpool.tile([1, NTOK], F32, tag="zrow")
                    if RECIP_MODE == "approx":
                        nc.vector.reciprocal_approx_fast(out=zrow[:], in_=ops[HD:HD + 1, 0:NTOK])
                    elif RECIP_MODE == "lnexp":
                        zl = zpool.tile([1, NTOK], F32, tag="zl")
                        nc.scalar.activation(out=zl[:], in_=ops[HD:HD + 1, 0:NTOK],
                                             func=AF.Ln, bias=0.0, scale=1.0)
                        nc.scalar.activation(out=zrow[:], in_=zl[:],
                                             func=AF.Exp, bias=0.0, scale=-1.0)
                    else:
                        nc.vector.reciprocal(out=zrow[:], in_=ops[HD:HD + 1, 0:NTOK])
                    zb = zpool.tile([64, NTOK], F32, tag="zb")
                    zap = zrow[:]
                    nc.sync.dma_start(zb[:], bass.AP(tensor=zap.tensor, offset=zap.offset,
                                                     ap=[zap.ap[0], [0, 64], zap.ap[1]]))
                    osl = ohat[r0:r0 + 64, g6, wcol:wcol + NTOK]
                    nc.vector.tensor_tensor(out=osl, in0=ops[0:64, 0:NTOK], in1=zb[:],
                                            op=OP.mult)

            # ---------- proj + residual ----------
            x1 = x1pool.tile([P, KT, PC2], BF16, tag="x1")
            for m in range(KT):
                pps = ps_mm.tile([P, PC2], F32, tag="mm")
                for k in range(KT):
                    nc.tensor.matmul(pps[:, :pc], lhsT=wp[:, k, m * P:(m + 1) * P],
                                     rhs=ohat[:, k, :pc], start=(k == 0), stop=(k == KT - 1))
                if pb is None:
                    nc.vector.tensor_tensor(out=x1[:, m, :pc], in0=pps[:, :pc],
                                            in1=xb[:, m, :pc], op=OP.add)
                else:
                    nc.vector.scalar_tensor_tensor(out=x1[:, m, :pc], in0=pps[:, :pc],
                                                   scalar=pb[:, m:m + 1],
                                                   in1=xb[:, m, :pc],
                                                   op0=OP.add, op1=OP.add)

            # ---------- LN2 + MLP ----------
            s1b, s2b = ln_stats(x1, KT, 1.0 / DIM)
            _, mu2b, rstd2b = ln_tail(s1b, s2b, 1.0 / DIM)
            h2 = ln_apply(x1, mu2b, rstd2b, "h2")

            g = gpool.tile([P, MT, PC2], BF16, tag="g")
            sg = ps_stat.tile([1, PC2], F32, tag="s1")
            ssg = ps_stat.tile([1, PC2], F32, tag="s2")
            for m in range(MT):
                p1 = ps_mm.tile([P, PC2], F32, tag="mm")
                for k in range(KT):
                    nc.tensor.matmul(p1[:, :pc], lhsT=w1[:, k, m * P:(m + 1) * P],
                                     rhs=h2[:, k, :pc], start=(k == 0), stop=(k == KT - 1))
                sf = mlppool.tile([P, PC2], BF16, tag="sf")
                if SILU_MODE == "silu":
                    nc.scalar.activation(out=sf[:, :pc], in_=p1[:, :pc], func=AF.Silu,
                                         bias=w1b[:, m:m + 1] if w1b is not None else 0.0,
                                         scale=1.0)
                else:
                    s1t = mlppool.tile([P, PC2], BF16, tag="s1t")
                    nc.scalar.activation(out=s1t[:, :pc], in_=p1[:, :pc], func=AF.Sigmoid,
                                         bias=w1b[:, m:m + 1] if w1b is not None else 0.0,
                                         scale=1.0)
                    nc.vector.scalar_tensor_tensor(
                        out=sf[:, :pc], in0=p1[:, :pc],
                        scalar=w1b[:, m:m + 1] if w1b is not None else 0.0,
                        in1=s1t[:, :pc], op0=OP.add, op1=OP.mult)
                p2 = ps_mm.tile([P, PC2], F32, tag="mm")
                for k in range(KT):
                    nc.tensor.matmul(p2[:, :pc], lhsT=w2[:, k, m * P:(m + 1) * P],
                                     rhs=h2[:, k, :pc], start=(k == 0), stop=(k == KT - 1))
                if w2b is None:
                    nc.vector.tensor_tensor(out=g[:, m, :pc], in0=p2[:, :pc],
                                            in1=sf[:, :pc], op=OP.mult)
                else:
                    nc.vector.scalar_tensor_tensor(out=g[:, m, :pc], in0=p2[:, :pc],
                                                   scalar=w2b[:, m:m + 1],
                                                   in1=sf[:, :pc],
                                                   op0=OP.add, op1=OP.mult)
                gsq = sqpool.tile([P, PC2], BF16, tag="xsq")
                nc.vector.tensor_tensor(out=gsq[:, :pc], in0=g[:, m, :pc],
                                        in1=g[:, m, :pc], op=OP.mult)
                nc.tensor.matmul(sg[:, :pc], lhsT=colones[:, 0:1], rhs=g[:, m, :pc],
                                 start=(m == 0), stop=(m == MT - 1), skip_group_check=True)
                nc.tensor.matmul(ssg[:, :pc], lhsT=colones[:, 0:1], rhs=gsq[:, :pc],
                                 start=(m == 0), stop=(m == MT - 1), skip_group_check=True)

            mu3r, _, rstd3b = ln_tail(sg, ssg, 1.0 / HID, want_bcast_mu=False)

            # ---------- w3 (streamed) + ln3-scale + residual -> out ----
            for m in range(KT):
                w3t = w3pool.tile([P, MT, P], BF16, tag="w3t")
                nc.sync.dma_start(w3t[:], w3d[:, :, m * P:(m + 1) * P])
                wps = ps_mm.tile([P, PC2], F32, tag="mm")
                for k in range(MT):
                    nc.tensor.matmul(wps[:, :pc], lhsT=w3t[:, k, :],
                                     rhs=g[:, k, :pc], start=(k == 0), stop=False,
                                     skip_group_check=True)
                # mean-centering of g folded in as a K=1 correction row
                nc.tensor.matmul(wps[:, :pc], lhsT=w3c[:, m * P:(m + 1) * P],
                                 rhs=mu3r[:, :pc], start=False, stop=True,
                                 skip_group_check=True)
                yt = ypool.tile([P, PC2], F32, tag="yt")
                nc.vector.tensor_tensor(out=yt[:, :pc], in0=wps[:, :pc],
                                        in1=rstd3b[:, :pc], op=OP.mult)
                if w3b is None:
                    nc.vector.tensor_tensor(out=yt[:, :pc], in0=yt[:, :pc],
                                            in1=x1[:, m, :pc], op=OP.add)
                else:
                    nc.vector.scalar_tensor_tensor(out=yt[:, :pc], in0=yt[:, :pc],
                                                   scalar=w3b[:, m:m + 1],
                                                   in1=x1[:, m, :pc],
                                                   op0=OP.add, op1=OP.add)
                nc.sync.dma_start(yT[:, m, c0:c0 + pc], yt[:, :pc])

    if loop_n > 1:
        with tc.For_i(0, loop_n, 1):
            emit_all_pairs()
    else:
        emit_all_pairs()


def _build(has_biases, nwin_total=NWIN, ncores=N_CORES, loop_n=1):
    key = ("prog", tuple(sorted(has_biases.items())), nwin_total, ncores, loop_n,
           SILU_MODE, RECIP_MODE, X_LOAD)
    if key in _cache:
        return _cache[key]
    nc = bacc.Bacc("TRN2", target_bir_lowering=False, debug=False,
                   enable_asserts=False, num_devices=ncores)
    toks = nwin_total * NTOK
    aps = {}
    if X_LOAD == "trunc":
        aps["xT"] = nc.dram_tensor("xT", [DIM, toks, 2], BF16, kind="ExternalInput").ap()
    else:
        aps["xT"] = nc.dram_tensor("xT", [DIM, toks], F32, kind="ExternalInput").ap()
    aps["yT"] = nc.dram_tensor("yT", [DIM, toks], F32, kind="ExternalOutput").ap()
    for nm, shp in [("wq", [DIM, DIM]), ("wk", [DIM, DIM]), ("wv", [DIM, DIM]),
                    ("wp", [DIM, DIM]), ("w1", [DIM, HID]), ("w2", [DIM, HID]),
                    ("w3", [HID, DIM])]:
        aps[nm] = nc.dram_tensor(nm, shp, BF16, kind="ExternalInput").ap()
    aps["cos2"] = nc.dram_tensor("cos2", [P, PC2], BF16, kind="ExternalInput").ap()
    aps["sin2"] = nc.dram_tensor("sin2", [P, PC2], BF16, kind="ExternalInput").ap()
    aps["r2t"] = nc.dram_tensor("r2t", [P, P], BF16, kind="ExternalInput").ap()
    aps["w3c"] = nc.dram_tensor("w3c", [1, DIM], BF16, kind="ExternalInput").ap()
    bias_specs = {"qb": DIM, "kb": DIM, "vb": DIM, "pb": DIM,
                  "w1b": HID, "w2b": HID, "w3b": DIM}
    for nm, d in bias_specs.items():
        if has_biases.get(nm):
            aps[nm] = nc.dram_tensor(nm, [d], F32, kind="ExternalInput").ap()
        else:
            aps[nm] = None
    if has_biases.get("vb"):
        aps["vbr"] = nc.dram_tensor("vbr", [1, DIM], BF16, kind="ExternalInput").ap()
    with tile.TileContext(nc) as tc:
        with ExitStack() as ctx:
            _emit(nc, tc, ctx, aps, has_biases, nwin_total, loop_n)
    nc.compile()
    _cache[key] = nc
    return nc


def _host_prep(inputs):
    f = {k: np.asarray(v, np.float32) if hasattr(v, "shape") else v
         for k, v in inputs.items()}
    scale = HD ** -0.5
    wq = f["ln1_w"][:, None] * f["q_w"] * scale
    wk = f["ln1_w"][:, None] * f["k_w"]
    wv = f["ln1_w"][:, None] * f["v_w"]
    qb = (f["ln1_b"] @ f["q_w"] + f["q_b"]) * scale
    kb = f["ln1_b"] @ f["k_w"]
    vb = f["ln1_b"] @ f["v_w"] + f["v_b"]
    wp = f["proj_w"]
    pb = f["proj_b"]
    w1 = f["ln2_w"][:, None] * f["w1_w"]
    w2 = f["ln2_w"][:, None] * f["w2_w"]
    w1b = f["ln2_b"] @ f["w1_w"] + f["w1_b"]
    w2b = f["ln2_b"] @ f["w2_w"] + f["w2_b"]
    w3 = f["ffn_w"][:, None] * f["w3_w"]
    w3b = f["ffn_b"] @ f["w3_w"] + f["w3_b"]
    w3c = -w3.sum(0).reshape(1, DIM)   # -colsum for mean-centering row

    cos, sin = _rope_tables()
    cosT = np.ascontiguousarray(cos.T)
    sinT = np.ascontiguousarray(sin.T)
    cos2 = np.tile(np.concatenate([cosT, cosT], 0), (1, 2))   # [128, 392]
    sin2 = np.tile(np.concatenate([sinT, sinT], 0), (1, 2))

    r = np.zeros((64, 64), np.float32)
    for i in range(32):
        r[2 * i, 2 * i + 1] = -1.0
        r[2 * i + 1, 2 * i] = 1.0
    r2 = np.zeros((128, 128), np.float32)
    r2[:64, :64] = r
    r2[64:, 64:] = r
    r2t = np.ascontiguousarray(r2.T)

    x = f["x"]
    pad = (-H) % WS
    nw = (H + pad) // WS
    xp = np.pad(x, ((0, 0), (0, pad), (0, pad), (0, 0)))
    t = xp.reshape(B, nw, WS, nw, WS, DIM).transpose(0, 1, 3, 2, 4, 5).reshape(B, NWIN * NTOK, DIM)

    shared = {
        "wq": wq.astype(BF16NP), "wk": wk.astype(BF16NP), "wv": wv.astype(BF16NP),
        "wp": wp.astype(BF16NP), "w1": w1.astype(BF16NP), "w2": w2.astype(BF16NP),
        "w3": w3.astype(BF16NP), "w3c": w3c.astype(BF16NP),
        "cos2": cos2.astype(BF16NP), "sin2": sin2.astype(BF16NP),
        "r2t": r2t.astype(BF16NP),
    }
    biases = {"qb": qb, "kb": kb, "vb": vb, "pb": pb, "w1b": w1b, "w2b": w2b, "w3b": w3b}
    has_biases = {k: bool(np.any(v != 0.0)) for k, v in biases.items()}
    for k, v in biases.items():
        if has_biases[k]:
            shared[k] = np.ascontiguousarray(v, np.float32)
    if has_biases["vb"]:
        shared["vbr"] = vb.reshape(1, DIM).astype(BF16NP)

    in_maps = []
    for b in range(B):
        m = dict(shared)
        xt32 = np.ascontiguousarray(t[b].T)     # [768, 4900] fp32
        if X_LOAD == "trunc":
            m["xT"] = xt32.view(BF16NP).reshape(DIM, TOKS, 2)
        else:
            m["xT"] = xt32
        in_maps.append(m)
    return in_maps, has_biases


def _host_post(results):
    pad = (-H) % WS
    nw = (H + pad) // WS
    Hp = H + pad
    y = np.empty((B, H, W, DIM), np.float32)
    for b in range(B):
        yb = np.asarray(results[b]["yT"])
        yw = yb.T.reshape(nw, nw, WS, WS, DIM).transpose(0, 2, 1, 3, 4).reshape(Hp, Hp, DIM)
        y[b] = yw[:H, :W, :]
    return y


def kernel(**inputs):
    in_maps, has_biases = _host_prep(inputs)
    nc = _build(has_biases)
    res = run_bass_kernel_spmd(nc, in_maps, core_ids=list(range(N_CORES)))
    return _host_post(res.results)


# revision 28
# speedup vs baseline: 1.0500x; 1.0500x over previous
"""Trainium2 Bass kernel for a Swin-style transformer block
(windowed attention with RoPE + SwiGLU MLP with sub-LN).

Sharding: data-parallel over batch B=8 -> one image per NeuronCore.
Each core computes the full block for its image in window-partitioned,
feature-major layout; the host does window (un)partitioning, LN-affine
folding into the projection weights, and RoPE table generation.

v2 design notes (vs the earlier gpsimd/DVE-heavy version):
- LN statistics via PE ones-matmuls into [1, pc] PSUM rows; partition
  broadcasts via K=1 ones-matmuls (no gpsimd partition_all_reduce).
- rstd = Exp(-0.5*Ln(var+eps)) on ScalarE so the activation table stays
  in natural_log_exp_and_others (shared with attention's Exp); only the
  MLP's Silu forces a table switch.
- Softmax denominators via reciprocal_approx_fast (5x faster than the
  iterative reciprocal).
- bf16 residual stream; all bulk DVE elementwise ops in bf16 (2x mode);
  ScalarE does the PSUM->SBUF evacuations.
- x loaded from fp32 DRAM as bf16 by DMAing the high 2 bytes
  (truncation); host emulation shows the extra error is ~1e-4.
"""
import numpy as np
import ml_dtypes
from contextlib import ExitStack

import concourse.bass as bass
import concourse.tile as tile
from concourse import bacc, mybir
from concourse.bass_utils import run_bass_kernel_spmd

BF16NP = ml_dtypes.bfloat16
F32 = mybir.dt.float32
BF16 = mybir.dt.bfloat16
OP = mybir.AluOpType
AF = mybir.ActivationFunctionType
AX = mybir.AxisListType

DIM = 768
HEADS = 12
HD = 64
HID = 2048
EPS = 1e-6
WS = 14
NTOK = WS * WS          # 196 tokens per window
B, H, W = 8, 64, 64
NWIN = 25               # 5x5 windows per image
TOKS = NWIN * NTOK      # 4900
KT = DIM // 128         # 6 feature tiles
MT = HID // 128         # 16 hid tiles
N_CORES = 8
P = 128
PC2 = 2 * NTOK          # 392: max columns per window-pair

_cache = {}
SILU_MODE = "expln"  # "expln" (all ACT funcs in one table set), "silu", "sigmoid"
RECIP_MODE = "lnexp"   # "lnexp" (ScalarE Exp(-Ln(z))), "approx", or "exact"
X_LOAD = "trunc"        # "trunc" (bf16 hi-bytes DMA) or "f32"


def _rope_tables():
    dim, pt, theta = 32, 16.0, 10000.0
    freqs = 1.0 / theta ** (np.arange(0, dim, 2, dtype=np.float32) / dim)
    f1 = np.repeat((np.arange(WS, dtype=np.float32) / WS * pt)[:, None] * freqs[None, :], 2, axis=-1)
    f = np.concatenate([
        np.broadcast_to(f1[:, None, :], (WS, WS, dim)),
        np.broadcast_to(f1[None, :, :], (WS, WS, dim)),
    ], -1).reshape(NTOK, 2 * dim)
    return np.cos(f), np.sin(f)   # [196, 64] fp32


def _emit(nc, tc, ctx, aps, has_b, nwin_total=NWIN, loop_n=1):
    pairs = []
    w = 0
    while w < nwin_total:
        pairs.append((w, w + 1) if w + 1 < nwin_total else (w,))
        w += 2

    # x DRAM view: [768, toks, 2] bf16 (fp32 reinterpreted); [..., 1] is
    # the high half = truncated bf16.
    if X_LOAD == "trunc":
        xTb = aps["xT"][:, :, 1:2].rearrange("(k p) n one -> p k (n one)", p=P)
    else:
        xT32 = aps["xT"].rearrange("(k p) n -> p k n", p=P)
    yT = aps["yT"].rearrange("(k p) n -> p k n", p=P)
    w3d = aps["w3"].rearrange("(k p) m -> p k m", p=P)    # [128, 16, 768]

    # Pin the activation table to natural_log_exp_and_others (id 6): it
    # covers every ACT func used (Exp/Ln/Copy/Identity/Square), so the
    # table-load pass inserts no further loads. Without this, Exp maps to
    # set 0 and Ln to set 5 and the pass thrashes (~2.7us per switch).
    nc.scalar.add_instruction(mybir.InstLoadActFuncSet(
        name=nc.get_next_instruction_name(), act_func_set_id=6, ins=[], outs=[]))

    consts = ctx.enter_context(tc.tile_pool(name="consts", bufs=1))
    wpool = ctx.enter_context(tc.tile_pool(name="weights", bufs=1))
    w3pool = ctx.enter_context(tc.tile_pool(name="w3s", bufs=2))
    xpool = ctx.enter_context(tc.tile_pool(name="x", bufs=2))
    sqpool = ctx.enter_context(tc.tile_pool(name="sq", bufs=2))
    rowpool = ctx.enter_context(tc.tile_pool(name="rows", bufs=1))
    bcpool = ctx.enter_context(tc.tile_pool(name="bc", bufs=2))
    hpool = ctx.enter_context(tc.tile_pool(name="h", bufs=1))
    tpool = ctx.enter_context(tc.tile_pool(name="tmp", bufs=2))
    qkpool = ctx.enter_context(tc.tile_pool(name="qk", bufs=2))
    ropepool = ctx.enter_context(tc.tile_pool(name="rope", bufs=2))
    vpool = ctx.enter_context(tc.tile_pool(name="v", bufs=2))
    epool = ctx.enter_context(tc.tile_pool(name="exp", bufs=2))
    zpool = ctx.enter_context(tc.tile_pool(name="z", bufs=2))
    opool = ctx.enter_context(tc.tile_pool(name="ohat", bufs=1))
    x1pool = ctx.enter_context(tc.tile_pool(name="x1", bufs=2))
    mlppool = ctx.enter_context(tc.tile_pool(name="mlp", bufs=2))
    gpool = ctx.enter_context(tc.tile_pool(name="g", bufs=1))
    ypool = ctx.enter_context(tc.tile_pool(name="y", bufs=2))

    ps_mm = ctx.enter_context(tc.tile_pool(name="psmm", bufs=2, space="PSUM"))
    ps_att = ctx.enter_context(tc.tile_pool(name="psatt", bufs=3, space="PSUM"))
    ps_rot = ctx.enter_context(tc.tile_pool(name="psrot", bufs=1, space="PSUM"))
    ps_stat = ctx.enter_context(tc.tile_pool(name="psstat", bufs=1, space="PSUM"))

    # --- constants / weights in SBUF ---
    def load_w(name, kdim, mdim):
        t = wpool.tile([P, kdim // P, mdim], BF16, tag=name)
        nc.sync.dma_start(t[:], aps[name].rearrange("(k p) m -> p k m", p=P))
        return t

    wq = load_w("wq", DIM, DIM)
    wk = load_w("wk", DIM, DIM)
    wv = load_w("wv", DIM, DIM)
    wp = load_w("wp", DIM, DIM)
    w1 = load_w("w1", DIM, HID)
    w2 = load_w("w2", DIM, HID)

    cos2 = consts.tile([P, PC2], BF16, tag="cos2")
    nc.sync.dma_start(cos2[:], aps["cos2"][:])
    sin2 = consts.tile([P, PC2], BF16, tag="sin2")
    nc.sync.dma_start(sin2[:], aps["sin2"][:])
    r2t = consts.tile([P, P], BF16, tag="r2t")
    nc.sync.dma_start(r2t[:], aps["r2t"][:])
    w3c = consts.tile([1, DIM], BF16, tag="w3c")
    nc.sync.dma_start(w3c[:], aps["w3c"][:])
    colones = consts.tile([P, 1], BF16, tag="colones")
    nc.vector.memset(colones[:], 1.0)
    rowones = consts.tile([1, P], BF16, tag="rowones")
    nc.vector.memset(rowones[:], 1.0)
    epsc = consts.tile([1, 1], F32, tag="epsc")
    nc.vector.memset(epsc[:], EPS)

    def bias_col(name, feat):
        if aps.get(name) is None:
            return None
        t = consts.tile([P, feat // P], F32, tag=name)
        nc.sync.dma_start(t[:], aps[name].rearrange("(k p) -> p k", p=P))
        return t

    qb = bias_col("qb", DIM)
    kb = bias_col("kb", DIM)
    vb = bias_col("vb", DIM)
    pb = bias_col("pb", DIM)
    w1b = bias_col("w1b", HID)
    w2b = bias_col("w2b", HID)
    w3b = bias_col("w3b", DIM)
    vbr = None
    if has_b.get("vb"):
        vbr = consts.tile([1, DIM], BF16, tag="vbr")
        nc.sync.dma_start(vbr[:], aps["vbr"][:])

    CHUNKS = [(0, P), (P, NTOK - P)]   # [128, 68] token chunks per window

    def emit_all_pairs():
        for wins in pairs:
            nwin = len(wins)
            pc = NTOK * nwin
            c0 = wins[0] * NTOK

            xb = xpool.tile([P, KT, PC2], BF16, tag="xb")
            if X_LOAD == "trunc":
                for k in range(KT):
                    nc.sync.dma_start(xb[:, k, :pc], xTb[:, k, c0:c0 + pc])
            else:
                x32 = xpool.tile([P, KT, PC2], F32, tag="x32")
                nc.sync.dma_start(x32[:, :, :pc], xT32[:, :, c0:c0 + pc])
                for k in range(KT):
                    nc.scalar.activation(out=xb[:, k, :pc], in_=x32[:, k, :pc],
                                         func=AF.Copy, bias=0.0, scale=1.0)

            # ---------- LN stats: sums via PE, tail via ScalarE+DVE ----
            def ln_stats(src, kt, inv_n):
                s1 = ps_stat.tile([1, PC2], F32, tag="s1")
                s2 = ps_stat.tile([1, PC2], F32, tag="s2")
                for k in range(kt):
                    xsq = sqpool.tile([P, PC2], BF16, tag="xsq")
                    nc.vector.tensor_tensor(out=xsq[:, :pc], in0=src[:, k, :pc],
                                            in1=src[:, k, :pc], op=OP.mult)
                    nc.tensor.matmul(s1[:, :pc], lhsT=colones[:, 0:1],
                                     rhs=src[:, k, :pc], start=(k == 0),
                                     stop=(k == kt - 1), skip_group_check=True)
                    nc.tensor.matmul(s2[:, :pc], lhsT=colones[:, 0:1],
                                     rhs=xsq[:, :pc], start=(k == 0),
                                     stop=(k == kt - 1), skip_group_check=True)
                return s1, s2

            def ln_tail(s1, s2, inv_n, want_bcast_mu=True):
                # mu = s1/n (bf16 row); var = s2/n - mu^2 (fp32 exact scale)
                mu_row = rowpool.tile([1, PC2], BF16, tag="mur")
                nc.scalar.activation(out=mu_row[:, :pc], in_=s1[:, :pc],
                                     func=AF.Identity, bias=0.0, scale=inv_n)
                msq = rowpool.tile([1, PC2], F32, tag="msq")
                nc.scalar.activation(out=msq[:, :pc], in_=s1[:, :pc],
                                     func=AF.Square, bias=0.0, scale=inv_n)
                varr = rowpool.tile([1, PC2], F32, tag="varr")
                nc.vector.scalar_tensor_tensor(out=varr[:, :pc], in0=s2[:, :pc],
                                               scalar=inv_n, in1=msq[:, :pc],
                                               op0=OP.mult, op1=OP.subtract)
                lnv = rowpool.tile([1, PC2], F32, tag="lnv")
                nc.scalar.activation(out=lnv[:, :pc], in_=varr[:, :pc],
                                     func=AF.Ln, bias=epsc[:], scale=1.0)
                rstd_row = rowpool.tile([1, PC2], BF16, tag="rstdr")
                nc.scalar.activation(out=rstd_row[:, :pc], in_=lnv[:, :pc],
                                     func=AF.Exp, bias=0.0, scale=-0.5)
                rsb_ps = ps_rot.tile([P, PC2], F32, tag="rot")
                nc.tensor.matmul(rsb_ps[:, :pc], lhsT=rowones[:, 0:P],
                                 rhs=rstd_row[:, :pc], start=True, stop=True)
                rstd_b = bcpool.tile([P, PC2], BF16, tag="rstdb")
                nc.scalar.activation(out=rstd_b[:, :pc], in_=rsb_ps[:, :pc],
                                     func=AF.Copy, bias=0.0, scale=1.0)
                mu_b = None
                if want_bcast_mu:
                    mub_ps = ps_rot.tile([P, PC2], F32, tag="rot")
                    nc.tensor.matmul(mub_ps[:, :pc], lhsT=rowones[:, 0:P],
                                     rhs=mu_row[:, :pc], start=True, stop=True)
                    mu_b = bcpool.tile([P, PC2], BF16, tag="mub")
                    nc.scalar.activation(out=mu_b[:, :pc], in_=mub_ps[:, :pc],
                                         func=AF.Copy, bias=0.0, scale=1.0)
                return mu_row, mu_b, rstd_b

            def ln_apply(src, mu_b, rstd_b, tag):
                hh = hpool.tile([P, KT, PC2], BF16, tag=tag)
                for k in range(KT):
                    tmpc = tpool.tile([P, PC2], BF16, tag="tmpc")
                    nc.vector.tensor_tensor(out=tmpc[:, :pc], in0=src[:, k, :pc],
                                            in1=mu_b[:, :pc], op=OP.subtract)
                    nc.vector.tensor_tensor(out=hh[:, k, :pc], in0=tmpc[:, :pc],
                                            in1=rstd_b[:, :pc], op=OP.mult)
                return hh

            s1a, s2a = ln_stats(xb, KT, 1.0 / DIM)
            _, mu1b, rstd1b = ln_tail(s1a, s2a, 1.0 / DIM)
            h1 = ln_apply(xb, mu1b, rstd1b, "h1")

            # ---------- QKV + RoPE (feature-major q/k) ----------
            def emit_qk(wmat, bcol, dest):
                for m in range(KT):
                    ps = ps_mm.tile([P, PC2], F32, tag="mm")
                    for k in range(KT):
                        nc.tensor.matmul(ps[:, :pc], lhsT=wmat[:, k, m * P:(m + 1) * P],
                                         rhs=h1[:, k, :pc], start=(k == 0), stop=(k == KT - 1))
                    qs = ropepool.tile([P, PC2], BF16, tag="qs")
                    if bcol is None:
                        nc.scalar.activation(out=qs[:, :pc], in_=ps[:, :pc],
                                             func=AF.Copy, bias=0.0, scale=1.0)
                    else:
                        nc.scalar.activation(out=qs[:, :pc], in_=ps[:, :pc],
                                             func=AF.Identity, bias=bcol[:, m:m + 1],
                                             scale=1.0)
                    rot = ps_rot.tile([P, PC2], F32, tag="rot")
                    nc.tensor.matmul(rot[:, :pc], lhsT=r2t[:], rhs=qs[:, :pc],
                                     start=True, stop=True)
                    t1 = ropepool.tile([P, PC2], BF16, tag="t1")
                    nc.vector.tensor_tensor(out=t1[:, :pc], in0=qs[:, :pc],
                                            in1=cos2[:, :pc], op=OP.mult)
                    t2 = ropepool.tile([P, PC2], BF16, tag="t2")
                    nc.vector.tensor_tensor(out=t2[:, :pc], in0=rot[:, :pc],
                                            in1=sin2[:, :pc], op=OP.mult)
                    nc.vector.tensor_tensor(out=dest[:, m, :pc], in0=t1[:, :pc],
                                            in1=t2[:, :pc], op=OP.add)

            qhat = qkpool.tile([P, KT, PC2], BF16, tag="qhat")
            khat = qkpool.tile([P, KT, PC2], BF16, tag="khat")
            emit_qk(wq, qb, qhat)
            emit_qk(wk, kb, khat)

            # ---------- V (token-major, per window, with ones column) --
            v_ts = []
            for wi in range(nwin):
                wcol = wi * NTOK
                vt = []
                for ci, (cs, cn) in enumerate(CHUNKS):
                    v_t = vpool.tile([P, HEADS, HD + 1], BF16, tag=f"v{ci}")
                    nc.vector.memset(v_t[:, :, HD:HD + 1], 1.0)
                    for half in range(2):
                        nh = DIM // 2
                        vps = ps_mm.tile([P, PC2], F32, tag="mm")
                        for k in range(KT):
                            nc.tensor.matmul(vps[0:cn, 0:nh],
                                             lhsT=h1[:, k, wcol + cs:wcol + cs + cn],
                                             rhs=wv[:, k, half * nh:(half + 1) * nh],
                                             start=(k == 0), stop=False if vbr is not None else (k == KT - 1),
                                             skip_group_check=True)
                        if vbr is not None:
                            nc.tensor.matmul(vps[0:cn, 0:nh],
                                             lhsT=rowones[:, 0:cn],
                                             rhs=vbr[:, half * nh:(half + 1) * nh],
                                             start=False, stop=True, skip_group_check=True)
                        nc.scalar.activation(
                            out=v_t[0:cn, half * (HEADS // 2):(half + 1) * (HEADS // 2), 0:HD],
                            in_=vps[0:cn, 0:nh].rearrange("p (h d) -> p h d", d=HD),
                            func=AF.Copy, bias=0.0, scale=1.0)
                    vt.append(v_t)
                v_ts.append(vt)

            # ---------- attention per window/head ----------
            ohat = opool.tile([P, KT, PC2], BF16, tag="ohat")
            for wi in range(nwin):
                wcol = wi * NTOK
                for hh in range(HEADS):
                    r0 = 64 * (hh % 2)
                    g6 = hh // 2
                    qsl = qhat[r0:r0 + 64, g6, wcol:wcol + NTOK]
                    ksl = khat[r0:r0 + 64, g6, wcol:wcol + NTOK]
                    es = []
                    for ci, (cs, cn) in enumerate(CHUNKS):
                        sps = ps_att.tile([P, PC2], F32, tag="att")
                        nc.tensor.matmul(sps[0:cn, 0:NTOK], lhsT=ksl[:, cs:cs + cn],
                                         rhs=qsl, start=True, stop=True)
                        e = epool.tile([P, NTOK], BF16, tag=f"e{ci}")
                        nc.scalar.activation(out=e[0:cn, :], in_=sps[0:cn, 0:NTOK],
                                             func=AF.Exp, bias=0.0, scale=1.0)
                        es.append(e)
                    ops = ps_att.tile([P, PC2], F32, tag="att")
                    for ci, (cs, cn) in enumerate(CHUNKS):
                        nc.tensor.matmul(ops[0:HD + 1, 0:NTOK], lhsT=v_ts[wi][ci][0:cn, hh, :],
                                         rhs=es[ci][0:cn, :], start=(ci == 0), stop=(ci == 1))
                    zrow = zpool.tile([1, NTOK], F32, tag="zrow")
                    if RECIP_MODE == "approx":
                        nc.vector.reciprocal_approx_fast(out=zrow[:], in_=ops[HD:HD + 1, 0:NTOK])
                    elif RECIP_MODE == "lnexp":
                        zl = zpool.tile([1, NTOK], F32, tag="zl")
                        nc.scalar.activation(out=zl[:], in_=ops[HD:HD + 1, 0:NTOK],
                                             func=AF.Ln, bias=0.0, scale=1.0)
                        nc.scalar.activation(out=zrow[:], in_=zl[:],
                                             func=AF.Exp, bias=0.0, scale=-1.0)
                    else:
                        nc.vector.reciprocal(out=zrow[:], in_=ops[HD:HD + 1, 0:NTOK])
                    zb = zpool.tile([64, NTOK], F32, tag="zb")
                    zap = zrow[:]
                    nc.sync.dma_start(zb[:], bass.AP(tensor=zap.tensor, offset=zap.offset,
                                                     ap=[zap.ap[0], [0, 64], zap.ap[1]]))
                    osl = ohat[r0:r0 + 64, g6, wcol:wcol + NTOK]
                    nc.vector.tensor_tensor(out=osl, in0=ops[0:64, 0:NTOK], in1=zb[:],
                                            op=OP.mult)

            # ---------- proj + residual ----------
            x1 = x1pool.tile([P, KT, PC2], BF16, tag="x1")
            for m in range(KT):
                pps = ps_mm.tile([P, PC2], F32, tag="mm")
                for k in range(KT):
                    nc.tensor.matmul(pps[:, :pc], lhsT=wp[:, k, m * P:(m + 1) * P],
                                     rhs=ohat[:, k, :pc], start=(k == 0), stop=(k == KT - 1))
                if pb is None:
                    nc.vector.tensor_tensor(out=x1[:, m, :pc], in0=pps[:, :pc],
                                            in1=xb[:, m, :pc], op=OP.add)
                else:
                    nc.vector.scalar_tensor_tensor(out=x1[:, m, :pc], in0=pps[:, :pc],
                                                   scalar=pb[:, m:m + 1],
                                                   in1=xb[:, m, :pc],
                                                   op0=OP.add, op1=OP.add)

            # ---------- LN2 + MLP ----------
            s1b, s2b = ln_stats(x1, KT, 1.0 / DIM)
            _, mu2b, rstd2b = ln_tail(s1b, s2b, 1.0 / DIM)
            h2 = ln_apply(x1, mu2b, rstd2b, "h2")

            g = gpool.tile([P, MT, PC2], BF16, tag="g")
            sg = ps_stat.tile([1, PC2], F32, tag="s1")
            ssg = ps_stat.tile([1, PC2], F32, tag="s2")
            for m in range(MT):
                p1 = ps_mm.tile([P, PC2], F32, tag="mm")
                for k in range(KT):
                    nc.tensor.matmul(p1[:, :pc], lhsT=w1[:, k, m * P:(m + 1) * P],
                                     rhs=h2[:, k, :pc], start=(k == 0), stop=(k == KT - 1))
                sf = mlppool.tile([P, PC2], BF16, tag="sf")
                if SILU_MODE == "silu":
                    nc.scalar.activation(out=sf[:, :pc], in_=p1[:, :pc], func=AF.Silu,
                                         bias=w1b[:, m:m + 1] if w1b is not None else 0.0,
                                         scale=1.0)
                elif SILU_MODE == "expln" and w1b is None:
                    # sigma(p1) = exp(-ln(1 + exp(-p1))); all funcs in the
                    # pinned table set.
                    e1 = mlppool.tile([P, PC2], BF16, tag="e1")
                    nc.scalar.activation(out=e1[:, :pc], in_=p1[:, :pc], func=AF.Exp,
                                         bias=0.0, scale=-1.0)
                    dd = mlppool.tile([P, PC2], BF16, tag="dd")
                    nc.vector.tensor_scalar_add(out=dd[:, :pc], in0=e1[:, :pc],
                                                scalar1=1.0)
                    ll = mlppool.tile([P, PC2], F32, tag="ll")
                    nc.scalar.activation(out=ll[:, :pc], in_=dd[:, :pc], func=AF.Ln,
                                         bias=0.0, scale=1.0)
                    ss = mlppool.tile([P, PC2], BF16, tag="ss")
                    nc.scalar.activation(out=ss[:, :pc], in_=ll[:, :pc], func=AF.Exp,
                                         bias=0.0, scale=-1.0)
                    if w1b is None:
                        nc.vector.tensor_tensor(out=sf[:, :pc], in0=p1[:, :pc],
                                                in1=ss[:, :pc], op=OP.mult)
                    else:
                        nc.vector.scalar_tensor_tensor(
                            out=sf[:, :pc], in0=p1[:, :pc],
                            scalar=w1b[:, m:m + 1],
                            in1=ss[:, :pc], op0=OP.add, op1=OP.mult)
                else:
                    s1t = mlppool.tile([P, PC2], BF16, tag="s1t")
                    nc.scalar.activation(out=s1t[:, :pc], in_=p1[:, :pc], func=AF.Sigmoid,
                                         bias=w1b[:, m:m + 1] if w1b is not None else 0.0,
                                         scale=1.0)
                    nc.vector.scalar_tensor_tensor(
                        out=sf[:, :pc], in0=p1[:, :pc],
                        scalar=w1b[:, m:m + 1] if w1b is not None else 0.0,
                        in1=s1t[:, :pc], op0=OP.add, op1=OP.mult)
                p2 = ps_mm.tile([P, PC2], F32, tag="mm")
                for k in range(KT):
                    nc.tensor.matmul(p2[:, :pc], lhsT=w2[:, k, m * P:(m + 1) * P],
                                     rhs=h2[:, k, :pc], start=(k == 0), stop=(k == KT - 1))
                if w2b is None:
                    nc.vector.tensor_tensor(out=g[:, m, :pc], in0=p2[:, :pc],
                                            in1=sf[:, :pc], op=OP.mult)
                else:
                    nc.vector.scalar_tensor_tensor(out=g[:, m, :pc], in0=p2[:, :pc],
                                                   scalar=w2b[:, m:m + 1],
                                                   in1=sf[:, :pc],
                                                   op0=OP.add, op1=OP.mult)
                gsq = sqpool.tile([P, PC2], BF16, tag="xsq")
                nc.vector.tensor_tensor(out=gsq[:, :pc], in0=g[:, m, :pc],
                                        in1=g[:, m, :pc], op=OP.mult)
                nc.tensor.matmul(sg[:, :pc], lhsT=colones[:, 0:1], rhs=g[:, m, :pc],
                                 start=(m == 0), stop=(m == MT - 1), skip_group_check=True)
                nc.tensor.matmul(ssg[:, :pc], lhsT=colones[:, 0:1], rhs=gsq[:, :pc],
                                 start=(m == 0), stop=(m == MT - 1), skip_group_check=True)

            mu3r, _, rstd3b = ln_tail(sg, ssg, 1.0 / HID, want_bcast_mu=False)

            # ---------- w3 (streamed) + ln3-scale + residual -> out ----
            for m in range(KT):
                w3t = w3pool.tile([P, MT, P], BF16, tag="w3t")
                nc.sync.dma_start(w3t[:], w3d[:, :, m * P:(m + 1) * P])
                wps = ps_mm.tile([P, PC2], F32, tag="mm")
                for k in range(MT):
                    nc.tensor.matmul(wps[:, :pc], lhsT=w3t[:, k, :],
                                     rhs=g[:, k, :pc], start=(k == 0), stop=False,
                                     skip_group_check=True)
                # mean-centering of g folded in as a K=1 correction row
                nc.tensor.matmul(wps[:, :pc], lhsT=w3c[:, m * P:(m + 1) * P],
                                 rhs=mu3r[:, :pc], start=False, stop=True,
                                 skip_group_check=True)
                yt = ypool.tile([P, PC2], F32, tag="yt")
                nc.vector.tensor_tensor(out=yt[:, :pc], in0=wps[:, :pc],
                                        in1=rstd3b[:, :pc], op=OP.mult)
                if w3b is None:
                    nc.vector.tensor_tensor(out=yt[:, :pc], in0=yt[:, :pc],
                                            in1=x1[:, m, :pc], op=OP.add)
                else:
                    nc.vector.scalar_tensor_tensor(out=yt[:, :pc], in0=yt[:, :pc],
                                                   scalar=w3b[:, m:m + 1],
                                                   in1=x1[:, m, :pc],
                                                   op0=OP.add, op1=OP.add)
                nc.sync.dma_start(yT[:, m, c0:c0 + pc], yt[:, :pc])

    if loop_n > 1:
        with tc.For_i(0, loop_n, 1):
            emit_all_pairs()
    else:
        emit_all_pairs()


def _build(has_biases, nwin_total=NWIN, ncores=N_CORES, loop_n=1):
    key = ("prog", tuple(sorted(has_biases.items())), nwin_total, ncores, loop_n,
           SILU_MODE, RECIP_MODE, X_LOAD)
    if key in _cache:
        return _cache[key]
    nc = bacc.Bacc("TRN2", target_bir_lowering=False, debug=False,
                   enable_asserts=False, num_devices=ncores)
    toks = nwin_total * NTOK
    aps = {}
    if X_LOAD == "trunc":
        aps["xT"] = nc.dram_tensor("xT", [DIM, toks, 2], BF16, kind="ExternalInput").ap()
    else:
        aps["xT"] = nc.dram_tensor("xT", [DIM, toks], F32, kind="ExternalInput").ap()
    aps["yT"] = nc.dram_tensor("yT", [DIM, toks], F32, kind="ExternalOutput").ap()
    for nm, shp in [("wq", [DIM, DIM]), ("wk", [DIM, DIM]), ("wv", [DIM, DIM]),
                    ("wp", [DIM, DIM]), ("w1", [DIM, HID]), ("w2", [DIM, HID]),
                    ("w3", [HID, DIM])]:
        aps[nm] = nc.dram_tensor(nm, shp, BF16, kind="ExternalInput").ap()
    aps["cos2"] = nc.dram_tensor("cos2", [P, PC2], BF16, kind="ExternalInput").ap()
    aps["sin2"] = nc.dram_tensor("sin2", [P, PC2], BF16, kind="ExternalInput").ap()
    aps["r2t"] = nc.dram_tensor("r2t", [P, P], BF16, kind="ExternalInput").ap()
    aps["w3c"] = nc.dram_tensor("w3c", [1, DIM], BF16, kind="ExternalInput").ap()
    bias_specs = {"qb": DIM, "kb": DIM, "vb": DIM, "pb": DIM,
                  "w1b": HID, "w2b": HID, "w3b": DIM}
    for nm, d in bias_specs.items():
        if has_biases.get(nm):
            aps[nm] = nc.dram_tensor(nm, [d], F32, kind="ExternalInput").ap()
        else:
            aps[nm] = None
    if has_biases.get("vb"):
        aps["vbr"] = nc.dram_tensor("vbr", [1, DIM], BF16, kind="ExternalInput").ap()
    with tile.TileContext(nc) as tc:
        with ExitStack() as ctx:
            _emit(nc, tc, ctx, aps, has_biases, nwin_total, loop_n)
    nc.compile()
    _cache[key] = nc
    return nc


def _host_prep(inputs):
    f = {k: np.asarray(v, np.float32) if hasattr(v, "shape") else v
         for k, v in inputs.items()}
    scale = HD ** -0.5
    wq = f["ln1_w"][:, None] * f["q_w"] * scale
    wk = f["ln1_w"][:, None] * f["k_w"]
    wv = f["ln1_w"][:, None] * f["v_w"]
    qb = (f["ln1_b"] @ f["q_w"] + f["q_b"]) * scale
    kb = f["ln1_b"] @ f["k_w"]
    vb = f["ln1_b"] @ f["v_w"] + f["v_b"]
    wp = f["proj_w"]
    pb = f["proj_b"]
    w1 = f["ln2_w"][:, None] * f["w1_w"]
    w2 = f["ln2_w"][:, None] * f["w2_w"]
    w1b = f["ln2_b"] @ f["w1_w"] + f["w1_b"]
    w2b = f["ln2_b"] @ f["w2_w"] + f["w2_b"]
    w3 = f["ffn_w"][:, None] * f["w3_w"]
    w3b = f["ffn_b"] @ f["w3_w"] + f["w3_b"]
    w3c = -w3.sum(0).reshape(1, DIM)   # -colsum for mean-centering row

    cos, sin = _rope_tables()
    cosT = np.ascontiguousarray(cos.T)
    sinT = np.ascontiguousarray(sin.T)
    cos2 = np.tile(np.concatenate([cosT, cosT], 0), (1, 2))   # [128, 392]
    sin2 = np.tile(np.concatenate([sinT, sinT], 0), (1, 2))

    r = np.zeros((64, 64), np.float32)
    for i in range(32):
        r[2 * i, 2 * i + 1] = -1.0
        r[2 * i + 1, 2 * i] = 1.0
    r2 = np.zeros((128, 128), np.float32)
    r2[:64, :64] = r
    r2[64:, 64:] = r
    r2t = np.ascontiguousarray(r2.T)

    x = f["x"]
    pad = (-H) % WS
    nw = (H + pad) // WS
    xp = np.pad(x, ((0, 0), (0, pad), (0, pad), (0, 0)))
    t = xp.reshape(B, nw, WS, nw, WS, DIM).transpose(0, 1, 3, 2, 4, 5).reshape(B, NWIN * NTOK, DIM)

    shared = {
        "wq": wq.astype(BF16NP), "wk": wk.astype(BF16NP), "wv": wv.astype(BF16NP),
        "wp": wp.astype(BF16NP), "w1": w1.astype(BF16NP), "w2": w2.astype(BF16NP),
        "w3": w3.astype(BF16NP), "w3c": w3c.astype(BF16NP),
        "cos2": cos2.astype(BF16NP), "sin2": sin2.astype(BF16NP),
        "r2t": r2t.astype(BF16NP),
    }
    biases = {"qb": qb, "kb": kb, "vb": vb, "pb": pb, "w1b": w1b, "w2b": w2b, "w3b": w3b}
    has_biases = {k: bool(np.any(v != 0.0)) for k, v in biases.items()}
    for k, v in biases.items():
        if has_biases[k]:
            shared[k] = np.ascontiguousarray(v, np.float32)
    if has_biases["vb"]:
        shared["vbr"] = vb.reshape(1, DIM).astype(BF16NP)

    in_maps = []
    for b in range(B):
        m = dict(shared)
        xt32 = np.ascontiguousarray(t[b].T)     # [768, 4900] fp32
        if X_LOAD == "trunc":
            m["xT"] = xt32.view(BF16NP).reshape(DIM, TOKS, 2)
        else:
            m["xT"] = xt32
        in_maps.append(m)
    return in_maps, has_biases


def _host_post(results):
    pad = (-H) % WS
    nw = (H + pad) // WS
    Hp = H + pad
    y = np.empty((B, H, W, DIM), np.float32)
    for b in range(B):
        yb = np.asarray(results[b]["yT"])
        yw = yb.T.reshape(nw, nw, WS, WS, DIM).transpose(0, 2, 1, 3, 4).reshape(Hp, Hp, DIM)
        y[b] = yw[:H, :W, :]
    return y


def kernel(**inputs):
    in_maps, has_biases = _host_prep(inputs)
    nc = _build(has_biases)
    res = run_bass_kernel_spmd(nc, in_maps, core_ids=list(range(N_CORES)))
    return _host_post(res.results)


# revision 34
# speedup vs baseline: 1.9030x; 1.8125x over previous
"""Trainium2 Bass kernel for a Swin-style transformer block
(windowed attention with RoPE + SwiGLU MLP with sub-LN).

Sharding: data-parallel over batch B=8 -> one image per NeuronCore.
Each core computes the full block for its image in window-partitioned,
feature-major layout; the host does window (un)partitioning, LN-affine
folding into the projection weights, and RoPE table generation.

v2 design notes (vs the earlier gpsimd/DVE-heavy version):
- LN statistics via PE ones-matmuls into [1, pc] PSUM rows; partition
  broadcasts via K=1 ones-matmuls (no gpsimd partition_all_reduce).
- rstd = Exp(-0.5*Ln(var+eps)) on ScalarE so the activation table stays
  in natural_log_exp_and_others (shared with attention's Exp); only the
  MLP's Silu forces a table switch.
- Softmax denominators via reciprocal_approx_fast (5x faster than the
  iterative reciprocal).
- bf16 residual stream; all bulk DVE elementwise ops in bf16 (2x mode);
  ScalarE does the PSUM->SBUF evacuations.
- x loaded from fp32 DRAM as bf16 by DMAing the high 2 bytes
  (truncation); host emulation shows the extra error is ~1e-4.
"""
import numpy as np
import ml_dtypes
from contextlib import ExitStack

import concourse.bass as bass
import concourse.tile as tile
from concourse import bacc, mybir
from concourse.bass_utils import run_bass_kernel_spmd

BF16NP = ml_dtypes.bfloat16
F32 = mybir.dt.float32
BF16 = mybir.dt.bfloat16
OP = mybir.AluOpType
AF = mybir.ActivationFunctionType
AX = mybir.AxisListType

DIM = 768
HEADS = 12
HD = 64
HID = 2048
EPS = 1e-6
WS = 14
NTOK = WS * WS          # 196 tokens per window
B, H, W = 8, 64, 64
NWIN = 25               # 5x5 windows per image
TOKS = NWIN * NTOK      # 4900
KT = DIM // 128         # 6 feature tiles
MT = HID // 128         # 16 hid tiles
N_CORES = 8
P = 128
PC2 = 2 * NTOK          # 392: max columns per window-pair

_cache = {}
SILU_MODE = "expln"  # "expln" (all ACT funcs in one table set), "silu", "sigmoid"
RECIP_MODE = "lnexp"   # "lnexp" (ScalarE Exp(-Ln(z))), "approx", or "exact"
X_LOAD = "f32"          # "f32" (contiguous fp32 + ScalarE convert) or "trunc"
POOL_BUFS = {"sq": 2, "rows": 1, "h": 1, "qk": 2, "rope": 2, "exp": 2,
             "z": 2, "ohat": 1, "mlp": 2, "g": 1,
             "psmm": 2, "psatt": 3, "psrot": 1}


def _rope_tables():
    dim, pt, theta = 32, 16.0, 10000.0
    freqs = 1.0 / theta ** (np.arange(0, dim, 2, dtype=np.float32) / dim)
    f1 = np.repeat((np.arange(WS, dtype=np.float32) / WS * pt)[:, None] * freqs[None, :], 2, axis=-1)
    f = np.concatenate([
        np.broadcast_to(f1[:, None, :], (WS, WS, dim)),
        np.broadcast_to(f1[None, :, :], (WS, WS, dim)),
    ], -1).reshape(NTOK, 2 * dim)
    return np.cos(f), np.sin(f)   # [196, 64] fp32


def _emit(nc, tc, ctx, aps, has_b, nwin_total=NWIN, loop_n=1):
    pairs = []
    w = 0
    while w < nwin_total:
        pairs.append((w, w + 1) if w + 1 < nwin_total else (w,))
        w += 2

    # x DRAM view: [768, toks, 2] bf16 (fp32 reinterpreted); [..., 1] is
    # the high half = truncated bf16.
    if X_LOAD == "trunc":
        xTb = aps["xT"][:, :, 1:2].rearrange("(k p) n one -> p k (n one)", p=P)
    else:
        xT32 = aps["xT"].rearrange("(k p) n -> p k n", p=P)
    yT = aps["yT"].rearrange("(k p) n -> p k n", p=P)
    w3d = aps["w3"].rearrange("(k p) m -> p k m", p=P)    # [128, 16, 768]

    # Pin the activation table to natural_log_exp_and_others (id 6): it
    # covers every ACT func used (Exp/Ln/Copy/Identity/Square), so the
    # table-load pass inserts no further loads. Without this, Exp maps to
    # set 0 and Ln to set 5 and the pass thrashes (~2.7us per switch).
    nc.scalar.add_instruction(mybir.InstLoadActFuncSet(
        name=nc.get_next_instruction_name(), act_func_set_id=6, ins=[], outs=[]))

    PB = dict(POOL_BUFS)
    consts = ctx.enter_context(tc.tile_pool(name="consts", bufs=1))
    wpool = ctx.enter_context(tc.tile_pool(name="weights", bufs=1))
    w3pool = ctx.enter_context(tc.tile_pool(name="w3s", bufs=2))
    xpool = ctx.enter_context(tc.tile_pool(name="x", bufs=2))
    sqpool = ctx.enter_context(tc.tile_pool(name="sq", bufs=PB["sq"]))
    rowpool = ctx.enter_context(tc.tile_pool(name="rows", bufs=PB["rows"]))
    bcpool = ctx.enter_context(tc.tile_pool(name="bc", bufs=2))
    hpool = ctx.enter_context(tc.tile_pool(name="h", bufs=PB["h"]))
    tpool = ctx.enter_context(tc.tile_pool(name="tmp", bufs=2))
    qkpool = ctx.enter_context(tc.tile_pool(name="qk", bufs=PB["qk"]))
    ropepool = ctx.enter_context(tc.tile_pool(name="rope", bufs=PB["rope"]))
    vpool = ctx.enter_context(tc.tile_pool(name="v", bufs=2))
    epool = ctx.enter_context(tc.tile_pool(name="exp", bufs=PB["exp"]))
    zpool = ctx.enter_context(tc.tile_pool(name="z", bufs=PB["z"]))
    opool = ctx.enter_context(tc.tile_pool(name="ohat", bufs=PB["ohat"]))
    x1pool = ctx.enter_context(tc.tile_pool(name="x1", bufs=2))
    mlppool = ctx.enter_context(tc.tile_pool(name="mlp", bufs=PB["mlp"]))
    gpool = ctx.enter_context(tc.tile_pool(name="g", bufs=PB["g"]))
    ypool = ctx.enter_context(tc.tile_pool(name="y", bufs=2))

    ps_mm = ctx.enter_context(tc.tile_pool(name="psmm", bufs=PB["psmm"], space="PSUM"))
    ps_att = ctx.enter_context(tc.tile_pool(name="psatt", bufs=PB["psatt"], space="PSUM"))
    ps_rot = ctx.enter_context(tc.tile_pool(name="psrot", bufs=PB["psrot"], space="PSUM"))
    ps_stat = ctx.enter_context(tc.tile_pool(name="psstat", bufs=1, space="PSUM"))

    # --- constants / weights in SBUF ---
    def load_w(name, kdim, mdim):
        t = wpool.tile([P, kdim // P, mdim], BF16, tag=name)
        nc.sync.dma_start(t[:], aps[name].rearrange("(k p) m -> p k m", p=P))
        return t

    wq = load_w("wq", DIM, DIM)
    wk = load_w("wk", DIM, DIM)
    wv = load_w("wv", DIM, DIM)
    wp = load_w("wp", DIM, DIM)
    w1 = load_w("w1", DIM, HID)
    w2 = load_w("w2", DIM, HID)

    cos2 = consts.tile([P, PC2], BF16, tag="cos2")
    nc.sync.dma_start(cos2[:], aps["cos2"][:])
    sin2 = consts.tile([P, PC2], BF16, tag="sin2")
    nc.sync.dma_start(sin2[:], aps["sin2"][:])
    r2t = consts.tile([P, P], BF16, tag="r2t")
    nc.sync.dma_start(r2t[:], aps["r2t"][:])
    w3c = consts.tile([1, DIM], BF16, tag="w3c")
    nc.sync.dma_start(w3c[:], aps["w3c"][:])
    colones = consts.tile([P, 1], BF16, tag="colones")
    nc.vector.memset(colones[:], 1.0)
    rowones = consts.tile([1, P], BF16, tag="rowones")
    nc.vector.memset(rowones[:], 1.0)
    epsc = consts.tile([1, 1], F32, tag="epsc")
    nc.vector.memset(epsc[:], EPS)

    def bias_col(name, feat):
        if aps.get(name) is None:
            return None
        t = consts.tile([P, feat // P], F32, tag=name)
        nc.sync.dma_start(t[:], aps[name].rearrange("(k p) -> p k", p=P))
        return t

    qb = bias_col("qb", DIM)
    kb = bias_col("kb", DIM)
    vb = bias_col("vb", DIM)
    pb = bias_col("pb", DIM)
    w1b = bias_col("w1b", HID)
    w2b = bias_col("w2b", HID)
    w3b = bias_col("w3b", DIM)
    vbr = None
    if has_b.get("vb"):
        vbr = consts.tile([1, DIM], BF16, tag="vbr")
        nc.sync.dma_start(vbr[:], aps["vbr"][:])

    CHUNKS = [(0, P), (P, NTOK - P)]   # [128, 68] token chunks per window

    def emit_all_pairs():
        for wins in pairs:
            nwin = len(wins)
            pc = NTOK * nwin
            c0 = wins[0] * NTOK

            xb = xpool.tile([P, KT, PC2], BF16, tag="xb")
            if X_LOAD == "trunc":
                for k in range(KT):
                    nc.sync.dma_start(xb[:, k, :pc], xTb[:, k, c0:c0 + pc])
            else:
                for k in range(KT):
                    x32 = tpool.tile([P, PC2], F32, tag="x32k")
                    nc.sync.dma_start(x32[:, :pc], xT32[:, k, c0:c0 + pc])
                    nc.scalar.activation(out=xb[:, k, :pc], in_=x32[:, :pc],
                                         func=AF.Copy, bias=0.0, scale=1.0)

            # ---------- LN stats: sums via PE, tail via ScalarE+DVE ----
            def ln_stats(src, kt, inv_n):
                s1 = ps_stat.tile([1, PC2], F32, tag="s1")
                s2 = ps_stat.tile([1, PC2], F32, tag="s2")
                for k in range(kt):
                    xsq = sqpool.tile([P, PC2], BF16, tag="xsq")
                    nc.vector.tensor_tensor(out=xsq[:, :pc], in0=src[:, k, :pc],
                                            in1=src[:, k, :pc], op=OP.mult)
                    nc.tensor.matmul(s1[:, :pc], lhsT=colones[:, 0:1],
                                     rhs=src[:, k, :pc], start=(k == 0),
                                     stop=(k == kt - 1), skip_group_check=True)
                    nc.tensor.matmul(s2[:, :pc], lhsT=colones[:, 0:1],
                                     rhs=xsq[:, :pc], start=(k == 0),
                                     stop=(k == kt - 1), skip_group_check=True)
                return s1, s2

            def ln_tail(s1, s2, inv_n, want_bcast_mu=True):
                # mu = s1/n (bf16 row); var = s2/n - mu^2 (fp32 exact scale)
                mu_row = rowpool.tile([1, PC2], BF16, tag="mur")
                nc.scalar.activation(out=mu_row[:, :pc], in_=s1[:, :pc],
                                     func=AF.Identity, bias=0.0, scale=inv_n)
                msq = rowpool.tile([1, PC2], F32, tag="msq")
                nc.scalar.activation(out=msq[:, :pc], in_=s1[:, :pc],
                                     func=AF.Square, bias=0.0, scale=inv_n)
                varr = rowpool.tile([1, PC2], F32, tag="varr")
                nc.vector.scalar_tensor_tensor(out=varr[:, :pc], in0=s2[:, :pc],
                                               scalar=inv_n, in1=msq[:, :pc],
                                               op0=OP.mult, op1=OP.subtract)
                lnv = rowpool.tile([1, PC2], F32, tag="lnv")
                nc.scalar.activation(out=lnv[:, :pc], in_=varr[:, :pc],
                                     func=AF.Ln, bias=epsc[:], scale=1.0)
                rstd_row = rowpool.tile([1, PC2], BF16, tag="rstdr")
                nc.scalar.activation(out=rstd_row[:, :pc], in_=lnv[:, :pc],
                                     func=AF.Exp, bias=0.0, scale=-0.5)
                rsb_ps = ps_rot.tile([P, PC2], F32, tag="rot")
                nc.tensor.matmul(rsb_ps[:, :pc], lhsT=rowones[:, 0:P],
                                 rhs=rstd_row[:, :pc], start=True, stop=True)
                rstd_b = bcpool.tile([P, PC2], BF16, tag="rstdb")
                nc.scalar.activation(out=rstd_b[:, :pc], in_=rsb_ps[:, :pc],
                                     func=AF.Copy, bias=0.0, scale=1.0)
                mu_b = None
                if want_bcast_mu:
                    mub_ps = ps_rot.tile([P, PC2], F32, tag="rot")
                    nc.tensor.matmul(mub_ps[:, :pc], lhsT=rowones[:, 0:P],
                                     rhs=mu_row[:, :pc], start=True, stop=True)
                    mu_b = bcpool.tile([P, PC2], BF16, tag="mub")
                    nc.scalar.activation(out=mu_b[:, :pc], in_=mub_ps[:, :pc],
                                         func=AF.Copy, bias=0.0, scale=1.0)
                return mu_row, mu_b, rstd_b

            def ln_apply(src, mu_b, rstd_b, tag):
                hh = hpool.tile([P, KT, PC2], BF16, tag=tag)
                for k in range(KT):
                    tmpc = tpool.tile([P, PC2], BF16, tag="tmpc")
                    nc.vector.tensor_tensor(out=tmpc[:, :pc], in0=src[:, k, :pc],
                                            in1=mu_b[:, :pc], op=OP.subtract)
                    nc.vector.tensor_tensor(out=hh[:, k, :pc], in0=tmpc[:, :pc],
                                            in1=rstd_b[:, :pc], op=OP.mult)
                return hh

            s1a, s2a = ln_stats(xb, KT, 1.0 / DIM)
            _, mu1b, rstd1b = ln_tail(s1a, s2a, 1.0 / DIM)
            h1 = ln_apply(xb, mu1b, rstd1b, "h1")

            # ---------- QKV + RoPE (feature-major q/k) ----------
            def emit_qk(wmat, bcol, dest):
                for m in range(KT):
                    ps = ps_mm.tile([P, PC2], F32, tag="mm")
                    for k in range(KT):
                        nc.tensor.matmul(ps[:, :pc], lhsT=wmat[:, k, m * P:(m + 1) * P],
                                         rhs=h1[:, k, :pc], start=(k == 0), stop=(k == KT - 1))
                    qs = ropepool.tile([P, PC2], BF16, tag="qs")
                    if bcol is None:
                        nc.scalar.activation(out=qs[:, :pc], in_=ps[:, :pc],
                                             func=AF.Copy, bias=0.0, scale=1.0)
                    else:
                        nc.scalar.activation(out=qs[:, :pc], in_=ps[:, :pc],
                                             func=AF.Identity, bias=bcol[:, m:m + 1],
                                             scale=1.0)
                    rot = ps_rot.tile([P, PC2], F32, tag="rot")
                    nc.tensor.matmul(rot[:, :pc], lhsT=r2t[:], rhs=qs[:, :pc],
                                     start=True, stop=True)
                    t1 = ropepool.tile([P, PC2], BF16, tag="t1")
                    nc.vector.tensor_tensor(out=t1[:, :pc], in0=qs[:, :pc],
                                            in1=cos2[:, :pc], op=OP.mult)
                    t2 = ropepool.tile([P, PC2], BF16, tag="t2")
                    nc.vector.tensor_tensor(out=t2[:, :pc], in0=rot[:, :pc],
                                            in1=sin2[:, :pc], op=OP.mult)
                    nc.vector.tensor_tensor(out=dest[:, m, :pc], in0=t1[:, :pc],
                                            in1=t2[:, :pc], op=OP.add)

            qhat = qkpool.tile([P, KT, PC2], BF16, tag="qhat")
            khat = qkpool.tile([P, KT, PC2], BF16, tag="khat")
            emit_qk(wq, qb, qhat)
            emit_qk(wk, kb, khat)

            # ---------- V (token-major, per window, with ones column) --
            v_ts = []
            for wi in range(nwin):
                wcol = wi * NTOK
                vt = []
                for ci, (cs, cn) in enumerate(CHUNKS):
                    v_t = vpool.tile([P, HEADS, HD + 1], BF16, tag=f"v{ci}")
                    nc.vector.memset(v_t[:, :, HD:HD + 1], 1.0)
                    for half in range(2):
                        nh = DIM // 2
                        vps = ps_mm.tile([P, PC2], F32, tag="mm")
                        for k in range(KT):
                            nc.tensor.matmul(vps[0:cn, 0:nh],
                                             lhsT=h1[:, k, wcol + cs:wcol + cs + cn],
                                             rhs=wv[:, k, half * nh:(half + 1) * nh],
                                             start=(k == 0), stop=False if vbr is not None else (k == KT - 1),
                                             skip_group_check=True)
                        if vbr is not None:
                            nc.tensor.matmul(vps[0:cn, 0:nh],
                                             lhsT=rowones[:, 0:cn],
                                             rhs=vbr[:, half * nh:(half + 1) * nh],
                                             start=False, stop=True, skip_group_check=True)
                        nc.scalar.activation(
                            out=v_t[0:cn, half * (HEADS // 2):(half + 1) * (HEADS // 2), 0:HD],
                            in_=vps[0:cn, 0:nh].rearrange("p (h d) -> p h d", d=HD),
                            func=AF.Copy, bias=0.0, scale=1.0)
                    vt.append(v_t)
                v_ts.append(vt)

            # ---------- attention per window/head ----------
            ohat = opool.tile([P, KT, PC2], BF16, tag="ohat")
            for wi in range(nwin):
                wcol = wi * NTOK
                for hh in range(HEADS):
                    r0 = 64 * (hh % 2)
                    g6 = hh // 2
                    qsl = qhat[r0:r0 + 64, g6, wcol:wcol + NTOK]
                    ksl = khat[r0:r0 + 64, g6, wcol:wcol + NTOK]
                    es = []
                    for ci, (cs, cn) in enumerate(CHUNKS):
                        sps = ps_att.tile([P, PC2], F32, tag="att")
                        nc.tensor.matmul(sps[0:cn, 0:NTOK], lhsT=ksl[:, cs:cs + cn],
                                         rhs=qsl, start=True, stop=True)
                        e = epool.tile([P, NTOK], BF16, tag=f"e{ci}")
                        nc.scalar.activation(out=e[0:cn, :], in_=sps[0:cn, 0:NTOK],
                                             func=AF.Exp, bias=0.0, scale=1.0)
                        es.append(e)
                    ops = ps_att.tile([P, PC2], F32, tag="att")
                    for ci, (cs, cn) in enumerate(CHUNKS):
                        nc.tensor.matmul(ops[0:HD + 1, 0:NTOK], lhsT=v_ts[wi][ci][0:cn, hh, :],
                                         rhs=es[ci][0:cn, :], start=(ci == 0), stop=(ci == 1))
                    zrow = zpool.tile([1, NTOK], BF16, tag="zrow")
                    if RECIP_MODE == "lnexp":
                        zl = zpool.tile([1, NTOK], F32, tag="zl")
                        nc.scalar.activation(out=zl[:], in_=ops[HD:HD + 1, 0:NTOK],
                                             func=AF.Ln, bias=0.0, scale=1.0)
                        nc.scalar.activation(out=zrow[:], in_=zl[:],
                                             func=AF.Exp, bias=0.0, scale=-1.0)
                    else:
                        with nc.allow_low_precision(reason="softmax denom bf16"):
                            nc.vector.reciprocal(out=zrow[:], in_=ops[HD:HD + 1, 0:NTOK])
                    # broadcast 1/z to 64 partitions on the PE (no DMA)
                    zb = ps_rot.tile([P, PC2], F32, tag="rot")
                    nc.tensor.matmul(zb[0:64, 0:NTOK], lhsT=rowones[:, 0:64],
                                     rhs=zrow[:], start=True, stop=True)
                    o_sb = zpool.tile([64, NTOK], BF16, tag="osb")
                    nc.scalar.activation(out=o_sb[:], in_=ops[0:64, 0:NTOK],
                                         func=AF.Copy, bias=0.0, scale=1.0)
                    osl = ohat[r0:r0 + 64, g6, wcol:wcol + NTOK]
                    nc.vector.tensor_tensor(out=osl, in0=o_sb[:], in1=zb[0:64, 0:NTOK],
                                            op=OP.mult)

            # ---------- proj + residual ----------
            x1 = x1pool.tile([P, KT, PC2], BF16, tag="x1")
            for m in range(KT):
                pps = ps_mm.tile([P, PC2], F32, tag="mm")
                for k in range(KT):
                    nc.tensor.matmul(pps[:, :pc], lhsT=wp[:, k, m * P:(m + 1) * P],
                                     rhs=ohat[:, k, :pc], start=(k == 0), stop=(k == KT - 1))
                if pb is None:
                    nc.vector.tensor_tensor(out=x1[:, m, :pc], in0=pps[:, :pc],
                                            in1=xb[:, m, :pc], op=OP.add)
                else:
                    nc.vector.scalar_tensor_tensor(out=x1[:, m, :pc], in0=pps[:, :pc],
                                                   scalar=pb[:, m:m + 1],
                                                   in1=xb[:, m, :pc],
                                                   op0=OP.add, op1=OP.add)

            # ---------- LN2 + MLP ----------
            s1b, s2b = ln_stats(x1, KT, 1.0 / DIM)
            _, mu2b, rstd2b = ln_tail(s1b, s2b, 1.0 / DIM)
            h2 = ln_apply(x1, mu2b, rstd2b, "h2")

            g = gpool.tile([P, MT, PC2], BF16, tag="g")
            sg = ps_stat.tile([1, PC2], F32, tag="s1")
            ssg = ps_stat.tile([1, PC2], F32, tag="s2")
            for m in range(MT):
                p1 = ps_mm.tile([P, PC2], F32, tag="mm")
                for k in range(KT):
                    nc.tensor.matmul(p1[:, :pc], lhsT=w1[:, k, m * P:(m + 1) * P],
                                     rhs=h2[:, k, :pc], start=(k == 0), stop=(k == KT - 1))
                sf = mlppool.tile([P, PC2], BF16, tag="sf")
                if SILU_MODE == "silu":
                    nc.scalar.activation(out=sf[:, :pc], in_=p1[:, :pc], func=AF.Silu,
                                         bias=w1b[:, m:m + 1] if w1b is not None else 0.0,
                                         scale=1.0)
                elif SILU_MODE == "expln" and w1b is None:
                    # sigma(p1) = exp(-ln(1 + exp(-p1))); all funcs in the
                    # pinned table set.
                    e1 = mlppool.tile([P, PC2], BF16, tag="e1")
                    nc.scalar.activation(out=e1[:, :pc], in_=p1[:, :pc], func=AF.Exp,
                                         bias=0.0, scale=-1.0)
                    dd = mlppool.tile([P, PC2], BF16, tag="dd")
                    nc.vector.tensor_scalar_add(out=dd[:, :pc], in0=e1[:, :pc],
                                                scalar1=1.0)
                    ll = mlppool.tile([P, PC2], F32, tag="ll")
                    nc.scalar.activation(out=ll[:, :pc], in_=dd[:, :pc], func=AF.Ln,
                                         bias=0.0, scale=1.0)
                    ss = mlppool.tile([P, PC2], BF16, tag="ss")
                    nc.scalar.activation(out=ss[:, :pc], in_=ll[:, :pc], func=AF.Exp,
                                         bias=0.0, scale=-1.0)
                    if w1b is None:
                        nc.vector.tensor_tensor(out=sf[:, :pc], in0=p1[:, :pc],
                                                in1=ss[:, :pc], op=OP.mult)
                    else:
                        nc.vector.scalar_tensor_tensor(
                            out=sf[:, :pc], in0=p1[:, :pc],
                            scalar=w1b[:, m:m + 1],
                            in1=ss[:, :pc], op0=OP.add, op1=OP.mult)
                else:
                    s1t = mlppool.tile([P, PC2], BF16, tag="s1t")
                    nc.scalar.activation(out=s1t[:, :pc], in_=p1[:, :pc], func=AF.Sigmoid,
                                         bias=w1b[:, m:m + 1] if w1b is not None else 0.0,
                                         scale=1.0)
                    nc.vector.scalar_tensor_tensor(
                        out=sf[:, :pc], in0=p1[:, :pc],
                        scalar=w1b[:, m:m + 1] if w1b is not None else 0.0,
                        in1=s1t[:, :pc], op0=OP.add, op1=OP.mult)
                p2 = ps_mm.tile([P, PC2], F32, tag="mm")
                for k in range(KT):
                    nc.tensor.matmul(p2[:, :pc], lhsT=w2[:, k, m * P:(m + 1) * P],
                                     rhs=h2[:, k, :pc], start=(k == 0), stop=(k == KT - 1))
                if w2b is None:
                    nc.vector.tensor_tensor(out=g[:, m, :pc], in0=p2[:, :pc],
                                            in1=sf[:, :pc], op=OP.mult)
                else:
                    nc.vector.scalar_tensor_tensor(out=g[:, m, :pc], in0=p2[:, :pc],
                                                   scalar=w2b[:, m:m + 1],
                                                   in1=sf[:, :pc],
                                                   op0=OP.add, op1=OP.mult)
                gsq = sqpool.tile([P, PC2], BF16, tag="xsq")
                nc.vector.tensor_tensor(out=gsq[:, :pc], in0=g[:, m, :pc],
                                        in1=g[:, m, :pc], op=OP.mult)
                nc.tensor.matmul(sg[:, :pc], lhsT=colones[:, 0:1], rhs=g[:, m, :pc],
                                 start=(m == 0), stop=(m == MT - 1), skip_group_check=True)
                nc.tensor.matmul(ssg[:, :pc], lhsT=colones[:, 0:1], rhs=gsq[:, :pc],
                                 start=(m == 0), stop=(m == MT - 1), skip_group_check=True)

            mu3r, _, rstd3b = ln_tail(sg, ssg, 1.0 / HID, want_bcast_mu=False)

            # ---------- w3 (streamed) + ln3-scale + residual -> out ----
            for m in range(KT):
                w3t = w3pool.tile([P, MT, P], BF16, tag="w3t")
                nc.sync.dma_start(w3t[:], w3d[:, :, m * P:(m + 1) * P])
                wps = ps_mm.tile([P, PC2], F32, tag="mm")
                for k in range(MT):
                    nc.tensor.matmul(wps[:, :pc], lhsT=w3t[:, k, :],
                                     rhs=g[:, k, :pc], start=(k == 0), stop=False,
                                     skip_group_check=True)
                # mean-centering of g folded in as a K=1 correction row
                nc.tensor.matmul(wps[:, :pc], lhsT=w3c[:, m * P:(m + 1) * P],
                                 rhs=mu3r[:, :pc], start=False, stop=True,
                                 skip_group_check=True)
                yt = ypool.tile([P, PC2], F32, tag="yt")
                nc.vector.tensor_tensor(out=yt[:, :pc], in0=wps[:, :pc],
                                        in1=rstd3b[:, :pc], op=OP.mult)
                if w3b is None:
                    nc.vector.tensor_tensor(out=yt[:, :pc], in0=yt[:, :pc],
                                            in1=x1[:, m, :pc], op=OP.add)
                else:
                    nc.vector.scalar_tensor_tensor(out=yt[:, :pc], in0=yt[:, :pc],
                                                   scalar=w3b[:, m:m + 1],
                                                   in1=x1[:, m, :pc],
                                                   op0=OP.add, op1=OP.add)
                nc.sync.dma_start(yT[:, m, c0:c0 + pc], yt[:, :pc])

    if loop_n > 1:
        with tc.For_i(0, loop_n, 1):
            emit_all_pairs()
    else:
        emit_all_pairs()


def _build(has_biases, nwin_total=NWIN, ncores=N_CORES, loop_n=1):
    key = ("prog", tuple(sorted(has_biases.items())), nwin_total, ncores, loop_n,
           SILU_MODE, RECIP_MODE, X_LOAD, tuple(sorted(POOL_BUFS.items())))
    if key in _cache:
        return _cache[key]
    nc = bacc.Bacc("TRN2", target_bir_lowering=False, debug=False,
                   enable_asserts=False, num_devices=ncores)
    toks = nwin_total * NTOK
    aps = {}
    if X_LOAD == "trunc":
        aps["xT"] = nc.dram_tensor("xT", [DIM, toks, 2], BF16, kind="ExternalInput").ap()
    else:
        aps["xT"] = nc.dram_tensor("xT", [DIM, toks], F32, kind="ExternalInput").ap()
    aps["yT"] = nc.dram_tensor("yT", [DIM, toks], F32, kind="ExternalOutput").ap()
    for nm, shp in [("wq", [DIM, DIM]), ("wk", [DIM, DIM]), ("wv", [DIM, DIM]),
                    ("wp", [DIM, DIM]), ("w1", [DIM, HID]), ("w2", [DIM, HID]),
                    ("w3", [HID, DIM])]:
        aps[nm] = nc.dram_tensor(nm, shp, BF16, kind="ExternalInput").ap()
    aps["cos2"] = nc.dram_tensor("cos2", [P, PC2], BF16, kind="ExternalInput").ap()
    aps["sin2"] = nc.dram_tensor("sin2", [P, PC2], BF16, kind="ExternalInput").ap()
    aps["r2t"] = nc.dram_tensor("r2t", [P, P], BF16, kind="ExternalInput").ap()
    aps["w3c"] = nc.dram_tensor("w3c", [1, DIM], BF16, kind="ExternalInput").ap()
    bias_specs = {"qb": DIM, "kb": DIM, "vb": DIM, "pb": DIM,
                  "w1b": HID, "w2b": HID, "w3b": DIM}
    for nm, d in bias_specs.items():
        if has_biases.get(nm):
            aps[nm] = nc.dram_tensor(nm, [d], F32, kind="ExternalInput").ap()
        else:
            aps[nm] = None
    if has_biases.get("vb"):
        aps["vbr"] = nc.dram_tensor("vbr", [1, DIM], BF16, kind="ExternalInput").ap()
    with tile.TileContext(nc) as tc:
        with ExitStack() as ctx:
            _emit(nc, tc, ctx, aps, has_biases, nwin_total, loop_n)
    nc.compile()
    _cache[key] = nc
    return nc


def _host_prep(inputs):
    f = {k: np.asarray(v, np.float32) if hasattr(v, "shape") else v
         for k, v in inputs.items()}
    scale = HD ** -0.5
    wq = f["ln1_w"][:, None] * f["q_w"] * scale
    wk = f["ln1_w"][:, None] * f["k_w"]
    wv = f["ln1_w"][:, None] * f["v_w"]
    qb = (f["ln1_b"] @ f["q_w"] + f["q_b"]) * scale
    kb = f["ln1_b"] @ f["k_w"]
    vb = f["ln1_b"] @ f["v_w"] + f["v_b"]
    wp = f["proj_w"]
    pb = f["proj_b"]
    w1 = f["ln2_w"][:, None] * f["w1_w"]
    w2 = f["ln2_w"][:, None] * f["w2_w"]
    w1b = f["ln2_b"] @ f["w1_w"] + f["w1_b"]
    w2b = f["ln2_b"] @ f["w2_w"] + f["w2_b"]
    w3 = f["ffn_w"][:, None] * f["w3_w"]
    w3b = f["ffn_b"] @ f["w3_w"] + f["w3_b"]
    w3c = -w3.sum(0).reshape(1, DIM)   # -colsum for mean-centering row

    cos, sin = _rope_tables()
    cosT = np.ascontiguousarray(cos.T)
    sinT = np.ascontiguousarray(sin.T)
    cos2 = np.tile(np.concatenate([cosT, cosT], 0), (1, 2))   # [128, 392]
    sin2 = np.tile(np.concatenate([sinT, sinT], 0), (1, 2))

    r = np.zeros((64, 64), np.float32)
    for i in range(32):
        r[2 * i, 2 * i + 1] = -1.0
        r[2 * i + 1, 2 * i] = 1.0
    r2 = np.zeros((128, 128), np.float32)
    r2[:64, :64] = r
    r2[64:, 64:] = r
    r2t = np.ascontiguousarray(r2.T)

    x = f["x"]
    pad = (-H) % WS
    nw = (H + pad) // WS
    xp = np.pad(x, ((0, 0), (0, pad), (0, pad), (0, 0)))
    t = xp.reshape(B, nw, WS, nw, WS, DIM).transpose(0, 1, 3, 2, 4, 5).reshape(B, NWIN * NTOK, DIM)

    shared = {
        "wq": wq.astype(BF16NP), "wk": wk.astype(BF16NP), "wv": wv.astype(BF16NP),
        "wp": wp.astype(BF16NP), "w1": w1.astype(BF16NP), "w2": w2.astype(BF16NP),
        "w3": w3.astype(BF16NP), "w3c": w3c.astype(BF16NP),
        "cos2": cos2.astype(BF16NP), "sin2": sin2.astype(BF16NP),
        "r2t": r2t.astype(BF16NP),
    }
    biases = {"qb": qb, "kb": kb, "vb": vb, "pb": pb, "w1b": w1b, "w2b": w2b, "w3b": w3b}
    has_biases = {k: bool(np.any(v != 0.0)) for k, v in biases.items()}
    for k, v in biases.items():
        if has_biases[k]:
            shared[k] = np.ascontiguousarray(v, np.float32)
    if has_biases["vb"]:
        shared["vbr"] = vb.reshape(1, DIM).astype(BF16NP)

    in_maps = []
    for b in range(B):
        m = dict(shared)
        xt32 = np.ascontiguousarray(t[b].T)     # [768, 4900] fp32
        if X_LOAD == "trunc":
            m["xT"] = xt32.view(BF16NP).reshape(DIM, TOKS, 2)
        else:
            m["xT"] = xt32
        in_maps.append(m)
    return in_maps, has_biases


def _host_post(results):
    pad = (-H) % WS
    nw = (H + pad) // WS
    Hp = H + pad
    y = np.empty((B, H, W, DIM), np.float32)
    for b in range(B):
        yb = np.asarray(results[b]["yT"])
        yw = yb.T.reshape(nw, nw, WS, WS, DIM).transpose(0, 2, 1, 3, 4).reshape(Hp, Hp, DIM)
        y[b] = yw[:H, :W, :]
    return y


def kernel(**inputs):
    in_maps, has_biases = _host_prep(inputs)
    nc = _build(has_biases)
    res = run_bass_kernel_spmd(nc, in_maps, core_ids=list(range(N_CORES)))
    return _host_post(res.results)


# revision 35
# speedup vs baseline: 1.9637x; 1.0319x over previous
"""Trainium2 Bass kernel for a Swin-style transformer block
(windowed attention with RoPE + SwiGLU MLP with sub-LN).

Sharding: data-parallel over batch B=8 -> one image per NeuronCore.
Each core computes the full block for its image in window-partitioned,
feature-major layout; the host does window (un)partitioning, LN-affine
folding into the projection weights, and RoPE table generation.

Numerics: bf16 matmul inputs with fp32 PSUM accumulation; fp32
residuals; LN1/LN2 statistics in fp32 (DVE tree-sum + GPSIMD partition
reduction), hid-LN statistics via PE ones-matmuls in bf16. Host
emulation of this scheme vs the fp32 reference: ~3.5e-3 absmax-rel.
"""
import numpy as np
import ml_dtypes
from contextlib import ExitStack

import concourse.bass as bass
import concourse.tile as tile
from concourse import bacc, mybir
from concourse import bass_isa
from concourse.bass_utils import run_bass_kernel_spmd

BF16NP = ml_dtypes.bfloat16
F32 = mybir.dt.float32
BF16 = mybir.dt.bfloat16
OP = mybir.AluOpType
AF = mybir.ActivationFunctionType
AX = mybir.AxisListType

DIM = 768
HEADS = 12
HD = 64
HID = 2048
EPS = 1e-6
WS = 14
NTOK = WS * WS          # 196 tokens per window
B, H, W = 8, 64, 64
NWIN = 25               # 5x5 windows per image
TOKS = NWIN * NTOK      # 4900
KT = DIM // 128         # 6 feature tiles
MT = HID // 128         # 16 hid tiles
N_CORES = 8
P = 128
PC2 = 2 * NTOK          # 392: max columns per window-pair

_cache = {}


def _rope_tables():
    dim, pt, theta = 32, 16.0, 10000.0
    freqs = 1.0 / theta ** (np.arange(0, dim, 2, dtype=np.float32) / dim)
    f1 = np.repeat((np.arange(WS, dtype=np.float32) / WS * pt)[:, None] * freqs[None, :], 2, axis=-1)
    f = np.concatenate([
        np.broadcast_to(f1[:, None, :], (WS, WS, dim)),
        np.broadcast_to(f1[None, :, :], (WS, WS, dim)),
    ], -1).reshape(NTOK, 2 * dim)
    return np.cos(f), np.sin(f)   # [196, 64] fp32


def _emit(nc, tc, ctx, aps, has_vb, nwin_total=NWIN, loop_n=1):
    pairs = []
    w = 0
    while w < nwin_total:
        pairs.append((w, w + 1) if w + 1 < nwin_total else (w,))
        w += 2

    xT = aps["xT"].rearrange("(k p) n -> p k n", p=P)     # [128, 6, TOKS]
    yT = aps["yT"].rearrange("(k p) n -> p k n", p=P)
    w3d = aps["w3"].rearrange("(k p) m -> p k m", p=P)    # [128, 16, 768]
    w2d = aps["w2"].rearrange("(k p) m -> p k m", p=P)    # [128, 6, 2048]

    consts = ctx.enter_context(tc.tile_pool(name="consts", bufs=1))
    wpool = ctx.enter_context(tc.tile_pool(name="weights", bufs=1))
    w3pool = ctx.enter_context(tc.tile_pool(name="w3s", bufs=2))
    xpool = ctx.enter_context(tc.tile_pool(name="x", bufs=2))
    tpool1 = ctx.enter_context(tc.tile_pool(name="sqtree", bufs=1))
    tpool2 = ctx.enter_context(tc.tile_pool(name="ctmp", bufs=1))
    lnpool = ctx.enter_context(tc.tile_pool(name="lnsmall", bufs=1))
    abpool = ctx.enter_context(tc.tile_pool(name="ab", bufs=2))
    hpool = ctx.enter_context(tc.tile_pool(name="h", bufs=1))
    ropepool = ctx.enter_context(tc.tile_pool(name="rope", bufs=2))
    qkpool = ctx.enter_context(tc.tile_pool(name="qk", bufs=2))
    vpool = ctx.enter_context(tc.tile_pool(name="v", bufs=2))
    epool = ctx.enter_context(tc.tile_pool(name="exp", bufs=2))
    opool = ctx.enter_context(tc.tile_pool(name="ohat", bufs=1))
    x1pool = ctx.enter_context(tc.tile_pool(name="x1", bufs=1))
    mlppool = ctx.enter_context(tc.tile_pool(name="mlp", bufs=2))
    gpool = ctx.enter_context(tc.tile_pool(name="g", bufs=1))
    ypool = ctx.enter_context(tc.tile_pool(name="y", bufs=2))
    zpool = ctx.enter_context(tc.tile_pool(name="z", bufs=2))

    ps_mm = ctx.enter_context(tc.tile_pool(name="psmm", bufs=2, space="PSUM"))
    ps_att = ctx.enter_context(tc.tile_pool(name="psatt", bufs=4, space="PSUM"))
    ps_stat = ctx.enter_context(tc.tile_pool(name="psstat", bufs=1, space="PSUM"))

    # --- constants / weights in SBUF ---
    def load_w(name, kdim, mdim):
        t = wpool.tile([P, kdim // P, mdim], BF16, tag=name)
        nc.sync.dma_start(t[:], aps[name].rearrange("(k p) m -> p k m", p=P))
        return t

    wq = load_w("wq", DIM, DIM)
    wk = load_w("wk", DIM, DIM)
    wv = load_w("wv", DIM, DIM)
    wp = load_w("wp", DIM, DIM)
    w1 = load_w("w1", DIM, HID)
    w2 = load_w("w2", DIM, HID)

    cos2 = consts.tile([P, PC2], BF16, tag="cos2")
    nc.sync.dma_start(cos2[:], aps["cos2"][:])
    sin2 = consts.tile([P, PC2], BF16, tag="sin2")
    nc.sync.dma_start(sin2[:], aps["sin2"][:])
    r2t = consts.tile([P, P], BF16, tag="r2t")
    nc.sync.dma_start(r2t[:], aps["r2t"][:])
    ones = consts.tile([P, P], BF16, tag="ones")
    nc.vector.memset(ones[:], 1.0)
    zcol = consts.tile([P, 1], F32, tag="zcol")
    nc.vector.memset(zcol[:], 0.0)
    eps1 = consts.tile([P, 1], F32, tag="eps1")
    nc.vector.memset(eps1[:], float(DIM) * float(DIM) * EPS)
    eps3 = consts.tile([P, 1], F32, tag="eps3")
    nc.vector.memset(eps3[:], float(HID) * float(HID) * EPS)

    def bias_col(name, feat):
        if aps.get(name) is None:
            return None
        t = consts.tile([P, feat // P], F32, tag=name)
        nc.sync.dma_start(t[:], aps[name].rearrange("(k p) -> p k", p=P))
        return t

    qb = bias_col("qb", DIM)
    kb = bias_col("kb", DIM)
    vb = bias_col("vb", DIM)
    pb = bias_col("pb", DIM)
    w1b = bias_col("w1b", HID)
    w2b = bias_col("w2b", HID)
    w3b = bias_col("w3b", DIM)

    def sc(bcol, m):
        return 0.0 if bcol is None else bcol[:, m:m + 1]

    CHUNKS = [(0, P), (P, NTOK - P)]   # [128, 68] token chunks per window

    def emit_all_pairs():
        for wins in pairs:
            nwin = len(wins)
            pc = NTOK * nwin
            c0 = wins[0] * NTOK

            x_t = xpool.tile([P, KT, PC2], F32, tag="x")
            nc.sync.dma_start(x_t[:, :, :pc], xT[:, :, c0:c0 + pc])

            # ---------- LN (feature-major, pure normalize) ----------
            def ln_stats_tree(src):
                # src [128, KT, pc] fp32 -> (sx, ssq) [1, pc] fp32
                xs = tpool1.tile([P, PC2], F32, tag="xs")
                nc.vector.tensor_tensor(out=xs[:, :pc], in0=src[:, 0, :pc], in1=src[:, 1, :pc], op=OP.add)
                for k in range(2, KT):
                    nc.vector.tensor_tensor(out=xs[:, :pc], in0=xs[:, :pc], in1=src[:, k, :pc], op=OP.add)
                sqa = tpool1.tile([P, PC2], F32, tag="sqa")
                nc.vector.tensor_tensor(out=sqa[:, :pc], in0=src[:, 0, :pc], in1=src[:, 0, :pc], op=OP.mult)
                for k in range(1, KT):
                    sqk = tpool1.tile([P, PC2], F32, tag="sqk")
                    nc.vector.tensor_tensor(out=sqk[:, :pc], in0=src[:, k, :pc], in1=src[:, k, :pc], op=OP.mult)
                    nc.vector.tensor_tensor(out=sqa[:, :pc], in0=sqa[:, :pc], in1=sqk[:, :pc], op=OP.add)
                sx = lnpool.tile([P, PC2], F32, tag="sx")
                nc.gpsimd.partition_all_reduce(sx[:, :pc], xs[:, :pc], channels=P,
                                               reduce_op=bass_isa.ReduceOp.add)
                ssq = lnpool.tile([P, PC2], F32, tag="ssq")
                nc.gpsimd.partition_all_reduce(ssq[:, :pc], sqa[:, :pc], channels=P,
                                               reduce_op=bass_isa.ReduceOp.add)
                return sx, ssq

            def ln_tail(sx, ssq, nfeat):
                # Inputs are partition-replicated sums [128, pc].
                # var*n^2 = n*ssq - sx^2 ; rstd = n / sqrt(var*n^2 + n^2 eps)
                t = lnpool.tile([P, PC2], F32, tag="t")
                nc.vector.tensor_tensor(out=t[:, :pc], in0=sx[:, :pc], in1=sx[:, :pc], op=OP.mult)
                nc.vector.scalar_tensor_tensor(out=t[:, :pc], in0=ssq[:, :pc], scalar=float(nfeat),
                                               in1=t[:, :pc], op0=OP.mult, op1=OP.subtract)
                nc.scalar.activation(out=t[:, :pc], in_=t[:, :pc], func=AF.Sqrt,
                                     bias=eps1[:] if nfeat == DIM else eps3[:], scale=1.0)
                nc.vector.reciprocal(out=t[:, :pc], in_=t[:, :pc])
                Ab = abpool.tile([P, PC2], F32, tag="absb")
                nc.scalar.activation(out=Ab[:, :pc], in_=t[:, :pc], func=AF.Copy,
                                     bias=0.0, scale=float(nfeat))
                Cb = abpool.tile([P, PC2], F32, tag="cbsb")
                nc.scalar.activation(out=Cb[:, :pc], in_=sx[:, :pc], func=AF.Copy,
                                     bias=0.0, scale=1.0 / float(nfeat))
                return Ab, Cb

            sx1, ssq1 = ln_stats_tree(x_t)
            Ab1, Cb1 = ln_tail(sx1, ssq1, DIM)
            h1 = hpool.tile([P, KT, PC2], BF16, tag="h1")
            for k in range(KT):
                tmpc = tpool2.tile([P, PC2], F32, tag="tmpc")
                nc.vector.tensor_tensor(out=tmpc[:, :pc], in0=x_t[:, k, :pc], in1=Cb1[:, :pc], op=OP.subtract)
                nc.vector.tensor_tensor(out=h1[:, k, :pc], in0=tmpc[:, :pc], in1=Ab1[:, :pc], op=OP.mult)

            # ---------- QKV + RoPE (feature-major q/k) ----------
            def emit_qk(wmat, bcol, dest):
                for m in range(KT):
                    ps = ps_mm.tile([P, PC2], F32, tag="mm")
                    for k in range(KT):
                        nc.tensor.matmul(ps[:, :pc], lhsT=wmat[:, k, m * P:(m + 1) * P],
                                         rhs=h1[:, k, :pc], start=(k == 0), stop=(k == KT - 1))
                    qs = ropepool.tile([P, PC2], BF16, tag="qs")
                    if bcol is None:
                        nc.vector.tensor_copy(out=qs[:, :pc], in_=ps[:, :pc])
                    else:
                        nc.vector.tensor_scalar_add(out=qs[:, :pc], in0=ps[:, :pc],
                                                    scalar1=bcol[:, m:m + 1])
                    rot = ps_mm.tile([P, PC2], F32, tag="mm")
                    nc.tensor.matmul(rot[:, :pc], lhsT=r2t[:], rhs=qs[:, :pc], start=True, stop=True)
                    t1 = ropepool.tile([P, PC2], BF16, tag="t1")
                    nc.vector.tensor_tensor(out=t1[:, :pc], in0=qs[:, :pc], in1=cos2[:, :pc], op=OP.mult)
                    t2 = ropepool.tile([P, PC2], BF16, tag="t2")
                    nc.vector.tensor_tensor(out=t2[:, :pc], in0=rot[:, :pc], in1=sin2[:, :pc], op=OP.mult)
                    nc.vector.tensor_tensor(out=dest[:, m, :pc], in0=t1[:, :pc], in1=t2[:, :pc], op=OP.add)

            qhat = qkpool.tile([P, KT, PC2], BF16, tag="qhat")
            khat = qkpool.tile([P, KT, PC2], BF16, tag="khat")
            emit_qk(wq, qb, qhat)
            emit_qk(wk, kb, khat)

            # ---------- V (token-major, per window, with ones column) ----------
            v_ts = []
            for wi in range(nwin):
                wcol = wi * NTOK
                vt = []
                for ci, (cs, cn) in enumerate(CHUNKS):
                    v_t = vpool.tile([P, HEADS, HD + 1], BF16, tag=f"v{ci}")
                    nc.vector.memset(v_t[:, :, HD:HD + 1], 1.0)
                    for half in range(2):
                        nh = DIM // 2
                        vps = ps_mm.tile([P, nh], F32, tag="mm")
                        for k in range(KT):
                            nc.tensor.matmul(vps[0:cn, :], lhsT=h1[:, k, wcol + cs:wcol + cs + cn],
                                             rhs=wv[:, k, half * nh:(half + 1) * nh],
                                             start=(k == 0), stop=(k == KT - 1))
                        nc.vector.tensor_copy(
                            out=v_t[0:cn, half * (HEADS // 2):(half + 1) * (HEADS // 2), 0:HD],
                            in_=vps[0:cn, :].rearrange("p (h d) -> p h d", d=HD))
                    vt.append(v_t)
                v_ts.append(vt)

            # ---------- attention per window/head ----------
            ohat = opool.tile([P, KT, PC2], BF16, tag="ohat")
            for wi in range(nwin):
                wcol = wi * NTOK
                for hh in range(HEADS):
                    r0 = 64 * (hh % 2)
                    g6 = hh // 2
                    qsl = qhat[r0:r0 + 64, g6, wcol:wcol + NTOK]
                    ksl = khat[r0:r0 + 64, g6, wcol:wcol + NTOK]
                    es = []
                    for ci, (cs, cn) in enumerate(CHUNKS):
                        sps = ps_att.tile([P, NTOK], F32, tag="att")
                        nc.tensor.matmul(sps[0:cn, :], lhsT=ksl[:, cs:cs + cn], rhs=qsl,
                                         start=True, stop=True)
                        e = epool.tile([P, NTOK], BF16, tag=f"e{ci}")
                        nc.scalar.activation(out=e[0:cn, :], in_=sps[0:cn, :], func=AF.Exp,
                                             bias=zcol[0:cn, :], scale=1.0)
                        es.append(e)
                    ops = ps_att.tile([P, NTOK], F32, tag="att")
                    for ci, (cs, cn) in enumerate(CHUNKS):
                        nc.tensor.matmul(ops[0:HD + 1, :], lhsT=v_ts[wi][ci][0:cn, hh, :],
                                         rhs=es[ci][0:cn, :], start=(ci == 0), stop=(ci == 1))
                    zrow = zpool.tile([1, NTOK], BF16, tag="zrow")
                    with nc.allow_low_precision(reason="softmax denom bf16 broadcast"):
                        nc.vector.reciprocal(out=zrow[:], in_=ops[HD:HD + 1, :])
                    zb = zpool.tile([64, NTOK], BF16, tag="zb")
                    zap = zrow[:]
                    nc.sync.dma_start(zb[:], bass.AP(tensor=zap.tensor, offset=zap.offset,
                                                     ap=[zap.ap[0], [0, 64], zap.ap[1]]))
                    osl = ohat[r0:r0 + 64, g6, wcol:wcol + NTOK]
                    nc.vector.tensor_tensor(out=osl, in0=ops[0:64, :], in1=zb[:], op=OP.mult)
                    if has_vb:
                        nc.vector.tensor_scalar_add(out=osl, in0=osl,
                                                    scalar1=vb[r0:r0 + 64, g6:g6 + 1])

            # ---------- proj + residual ----------
            x1 = x1pool.tile([P, KT, PC2], F32, tag="x1")
            for m in range(KT):
                pps = ps_mm.tile([P, PC2], F32, tag="mm")
                for k in range(KT):
                    nc.tensor.matmul(pps[:, :pc], lhsT=wp[:, k, m * P:(m + 1) * P],
                                     rhs=ohat[:, k, :pc], start=(k == 0), stop=(k == KT - 1))
                nc.vector.scalar_tensor_tensor(out=x1[:, m, :pc], in0=pps[:, :pc], scalar=sc(pb, m),
                                               in1=x_t[:, m, :pc], op0=OP.add, op1=OP.add)

            # ---------- LN2 + MLP ----------
            sx2, ssq2 = ln_stats_tree(x1)
            Ab2, Cb2 = ln_tail(sx2, ssq2, DIM)
            h2 = hpool.tile([P, KT, PC2], BF16, tag="h2")
            for k in range(KT):
                tmpc = tpool2.tile([P, PC2], F32, tag="tmpc")
                nc.vector.tensor_tensor(out=tmpc[:, :pc], in0=x1[:, k, :pc], in1=Cb2[:, :pc], op=OP.subtract)
                nc.vector.tensor_tensor(out=h2[:, k, :pc], in0=tmpc[:, :pc], in1=Ab2[:, :pc], op=OP.mult)

            g = gpool.tile([P, MT, PC2], BF16, tag="g")
            sg = ps_stat.tile([1, PC2], F32, tag="sg")
            ssg = ps_stat.tile([1, PC2], F32, tag="ssg")
            for m in range(MT):
                p1 = ps_mm.tile([P, PC2], F32, tag="mm")
                for k in range(KT):
                    nc.tensor.matmul(p1[:, :pc], lhsT=w1[:, k, m * P:(m + 1) * P],
                                     rhs=h2[:, k, :pc], start=(k == 0), stop=(k == KT - 1))
                s1 = mlppool.tile([P, PC2], BF16, tag="s1")
                nc.scalar.activation(out=s1[:, :pc], in_=p1[:, :pc], func=AF.Sigmoid,
                                     bias=w1b[:, m:m + 1] if w1b is not None else zcol[:],
                                     scale=1.0)
                sf = mlppool.tile([P, PC2], BF16, tag="sf")
                nc.vector.scalar_tensor_tensor(out=sf[:, :pc], in0=p1[:, :pc], scalar=sc(w1b, m),
                                               in1=s1[:, :pc], op0=OP.add, op1=OP.mult)
                p2 = ps_mm.tile([P, PC2], F32, tag="mm")
                for k in range(KT):
                    nc.tensor.matmul(p2[:, :pc], lhsT=w2[:, k, m * P:(m + 1) * P],
                                     rhs=h2[:, k, :pc], start=(k == 0), stop=(k == KT - 1))
                nc.vector.scalar_tensor_tensor(out=g[:, m, :pc], in0=p2[:, :pc], scalar=sc(w2b, m),
                                               in1=sf[:, :pc], op0=OP.add, op1=OP.mult)
                sqg = mlppool.tile([P, PC2], BF16, tag="sqg")
                nc.vector.tensor_tensor(out=sqg[:, :pc], in0=g[:, m, :pc], in1=g[:, m, :pc], op=OP.mult)
                nc.tensor.matmul(sg[:, :pc], lhsT=ones[:, 0:1], rhs=g[:, m, :pc],
                                 start=(m == 0), stop=(m == MT - 1), skip_group_check=True)
                nc.tensor.matmul(ssg[:, :pc], lhsT=ones[:, 0:1], rhs=sqg[:, :pc],
                                 start=(m == 0), stop=(m == MT - 1), skip_group_check=True)

            sg_sb = lnpool.tile([P, PC2], F32, tag="sgsb")
            nc.vector.tensor_copy(out=sg_sb[0:1, :pc], in_=sg[:, :pc])
            nc.gpsimd.partition_broadcast(sg_sb[:, :pc], sg_sb[0:1, :pc], channels=P)
            ssg_sb = lnpool.tile([P, PC2], F32, tag="ssgsb")
            nc.vector.tensor_copy(out=ssg_sb[0:1, :pc], in_=ssg[:, :pc])
            nc.gpsimd.partition_broadcast(ssg_sb[:, :pc], ssg_sb[0:1, :pc], channels=P)
            Ab3, Cb3 = ln_tail(sg_sb, ssg_sb, HID)

            for m in range(MT):
                nc.vector.tensor_tensor(out=g[:, m, :pc], in0=g[:, m, :pc], in1=Cb3[:, :pc], op=OP.subtract)

            # ---------- w3 (streamed) + ln3-scale + residual -> output ----------
            for m in range(KT):
                w3t = w3pool.tile([P, MT, P], BF16, tag="w3t")
                nc.sync.dma_start(w3t[:], w3d[:, :, m * P:(m + 1) * P])
                wps = ps_mm.tile([P, PC2], F32, tag="mm")
                for k in range(MT):
                    nc.tensor.matmul(wps[:, :pc], lhsT=w3t[:, k, :],
                                     rhs=g[:, k, :pc], start=(k == 0), stop=(k == MT - 1))
                yt = ypool.tile([P, PC2], F32, tag="yt")
                nc.vector.tensor_tensor(out=yt[:, :pc], in0=wps[:, :pc], in1=Ab3[:, :pc], op=OP.mult)
                nc.vector.scalar_tensor_tensor(out=yt[:, :pc], in0=yt[:, :pc], scalar=sc(w3b, m),
                                               in1=x1[:, m, :pc], op0=OP.add, op1=OP.add)
                nc.sync.dma_start(yT[:, m, c0:c0 + pc], yt[:, :pc])


    if loop_n > 1:
        with tc.For_i(0, loop_n, 1):
            emit_all_pairs()
    else:
        emit_all_pairs()


def _build(has_biases, nwin_total=NWIN, ncores=N_CORES, loop_n=1):
    key = ("prog", tuple(sorted(has_biases.items())), nwin_total, ncores, loop_n)
    if key in _cache:
        return _cache[key]
    nc = bacc.Bacc("TRN2", target_bir_lowering=False, debug=False,
                   enable_asserts=False, num_devices=ncores)
    toks = nwin_total * NTOK
    aps = {}
    aps["xT"] = nc.dram_tensor("xT", [DIM, toks], F32, kind="ExternalInput").ap()
    aps["yT"] = nc.dram_tensor("yT", [DIM, toks], F32, kind="ExternalOutput").ap()
    for nm, shp in [("wq", [DIM, DIM]), ("wk", [DIM, DIM]), ("wv", [DIM, DIM]),
                    ("wp", [DIM, DIM]), ("w1", [DIM, HID]), ("w2", [DIM, HID]),
                    ("w3", [HID, DIM])]:
        aps[nm] = nc.dram_tensor(nm, shp, BF16, kind="ExternalInput").ap()
    aps["cos2"] = nc.dram_tensor("cos2", [P, PC2], BF16, kind="ExternalInput").ap()
    aps["sin2"] = nc.dram_tensor("sin2", [P, PC2], BF16, kind="ExternalInput").ap()
    aps["r2t"] = nc.dram_tensor("r2t", [P, P], BF16, kind="ExternalInput").ap()
    bias_specs = {"qb": DIM, "kb": DIM, "vb": DIM, "pb": DIM,
                  "w1b": HID, "w2b": HID, "w3b": DIM}
    for nm, d in bias_specs.items():
        if has_biases.get(nm):
            aps[nm] = nc.dram_tensor(nm, [d], F32, kind="ExternalInput").ap()
        else:
            aps[nm] = None
    with tile.TileContext(nc) as tc:
        with ExitStack() as ctx:
            _emit(nc, tc, ctx, aps, bool(has_biases.get("vb")), nwin_total, loop_n)
    nc.compile()
    _cache[key] = nc
    return nc


def _host_prep(inputs):
    f = {k: np.asarray(v, np.float32) if hasattr(v, "shape") else v
         for k, v in inputs.items()}
    scale = HD ** -0.5
    wq = f["ln1_w"][:, None] * f["q_w"] * scale
    wk = f["ln1_w"][:, None] * f["k_w"]
    wv = f["ln1_w"][:, None] * f["v_w"]
    qb = (f["ln1_b"] @ f["q_w"] + f["q_b"]) * scale
    kb = f["ln1_b"] @ f["k_w"]
    vb = f["ln1_b"] @ f["v_w"] + f["v_b"]
    wp = f["proj_w"]
    pb = f["proj_b"]
    w1 = f["ln2_w"][:, None] * f["w1_w"]
    w2 = f["ln2_w"][:, None] * f["w2_w"]
    w1b = f["ln2_b"] @ f["w1_w"] + f["w1_b"]
    w2b = f["ln2_b"] @ f["w2_w"] + f["w2_b"]
    w3 = f["ffn_w"][:, None] * f["w3_w"]
    w3b = f["ffn_b"] @ f["w3_w"] + f["w3_b"]

    cos, sin = _rope_tables()
    cosT = np.ascontiguousarray(cos.T)
    sinT = np.ascontiguousarray(sin.T)
    cos2 = np.tile(np.concatenate([cosT, cosT], 0), (1, 2))   # [128, 392]
    sin2 = np.tile(np.concatenate([sinT, sinT], 0), (1, 2))

    r = np.zeros((64, 64), np.float32)
    for i in range(32):
        r[2 * i, 2 * i + 1] = -1.0
        r[2 * i + 1, 2 * i] = 1.0
    r2 = np.zeros((128, 128), np.float32)
    r2[:64, :64] = r
    r2[64:, 64:] = r
    r2t = np.ascontiguousarray(r2.T)

    x = f["x"]
    pad = (-H) % WS
    nw = (H + pad) // WS
    xp = np.pad(x, ((0, 0), (0, pad), (0, pad), (0, 0)))
    t = xp.reshape(B, nw, WS, nw, WS, DIM).transpose(0, 1, 3, 2, 4, 5).reshape(B, NWIN * NTOK, DIM)

    shared = {
        "wq": wq.astype(BF16NP), "wk": wk.astype(BF16NP), "wv": wv.astype(BF16NP),
        "wp": wp.astype(BF16NP), "w1": w1.astype(BF16NP), "w2": w2.astype(BF16NP),
        "w3": w3.astype(BF16NP),
        "cos2": cos2.astype(BF16NP), "sin2": sin2.astype(BF16NP),
        "r2t": r2t.astype(BF16NP),
    }
    biases = {"qb": qb, "kb": kb, "vb": vb, "pb": pb, "w1b": w1b, "w2b": w2b, "w3b": w3b}
    has_biases = {k: bool(np.any(v != 0.0)) for k, v in biases.items()}
    for k, v in biases.items():
        if has_biases[k]:
            shared[k] = np.ascontiguousarray(v, np.float32)

    in_maps = []
    for b in range(B):
        m = dict(shared)
        m["xT"] = np.ascontiguousarray(t[b].T)     # [768, 4900] fp32
        in_maps.append(m)
    return in_maps, has_biases


def _host_post(results):
    pad = (-H) % WS
    nw = (H + pad) // WS
    Hp = H + pad
    y = np.empty((B, H, W, DIM), np.float32)
    for b in range(B):
        yb = np.asarray(results[b]["yT"])
        yw = yb.T.reshape(nw, nw, WS, WS, DIM).transpose(0, 2, 1, 3, 4).reshape(Hp, Hp, DIM)
        y[b] = yw[:H, :W, :]
    return y


def kernel(**inputs):
    in_maps, has_biases = _host_prep(inputs)
    nc = _build(has_biases)
    res = run_bass_kernel_spmd(nc, in_maps, core_ids=list(range(N_CORES)))
    return _host_post(res.results)



# revision 36
# speedup vs baseline: 2.1832x; 1.1118x over previous
"""Trainium2 Bass kernel for a Swin-style transformer block
(windowed attention with RoPE + SwiGLU MLP with sub-LN).

Sharding: data-parallel over batch B=8 -> one image per NeuronCore.
Each core computes the full block for its image in window-partitioned,
feature-major layout; the host does window (un)partitioning, LN-affine
folding into the projection weights, and RoPE table generation.

v2 design notes (vs the earlier gpsimd/DVE-heavy version):
- LN statistics via PE ones-matmuls into [1, pc] PSUM rows; partition
  broadcasts via K=1 ones-matmuls (no gpsimd partition_all_reduce).
- rstd = Exp(-0.5*Ln(var+eps)) on ScalarE so the activation table stays
  in natural_log_exp_and_others (shared with attention's Exp); only the
  MLP's Silu forces a table switch.
- Softmax denominators via reciprocal_approx_fast (5x faster than the
  iterative reciprocal).
- bf16 residual stream; all bulk DVE elementwise ops in bf16 (2x mode);
  ScalarE does the PSUM->SBUF evacuations.
- x loaded from fp32 DRAM as bf16 by DMAing the high 2 bytes
  (truncation); host emulation shows the extra error is ~1e-4.
"""
import numpy as np
import ml_dtypes
from contextlib import ExitStack

import concourse.bass as bass
import concourse.tile as tile
from concourse import bacc, mybir
from concourse.bass_utils import run_bass_kernel_spmd

BF16NP = ml_dtypes.bfloat16
F32 = mybir.dt.float32
BF16 = mybir.dt.bfloat16
OP = mybir.AluOpType
AF = mybir.ActivationFunctionType
AX = mybir.AxisListType

DIM = 768
HEADS = 12
HD = 64
HID = 2048
EPS = 1e-6
WS = 14
NTOK = WS * WS          # 196 tokens per window
B, H, W = 8, 64, 64
NWIN = 25               # 5x5 windows per image
TOKS = NWIN * NTOK      # 4900
KT = DIM // 128         # 6 feature tiles
MT = HID // 128         # 16 hid tiles
N_CORES = 8
P = 128
PC2 = 2 * NTOK          # 392: max columns per window-pair

_cache = {}
SILU_MODE = "expln"  # "expln" (all ACT funcs in one table set), "silu", "sigmoid"
RECIP_MODE = "lnexp"   # "lnexp" (ScalarE Exp(-Ln(z))), "approx", or "exact"
X_LOAD = "f32"          # "f32" (contiguous fp32 + ScalarE convert) or "trunc"
POOL_BUFS = {"sq": 2, "rows": 1, "h": 1, "qk": 2, "rope": 2, "exp": 2,
             "z": 2, "ohat": 1, "mlp": 2, "g": 1,
             "psmm": 2, "psatt": 3, "psrot": 1}


def _rope_tables():
    dim, pt, theta = 32, 16.0, 10000.0
    freqs = 1.0 / theta ** (np.arange(0, dim, 2, dtype=np.float32) / dim)
    f1 = np.repeat((np.arange(WS, dtype=np.float32) / WS * pt)[:, None] * freqs[None, :], 2, axis=-1)
    f = np.concatenate([
        np.broadcast_to(f1[:, None, :], (WS, WS, dim)),
        np.broadcast_to(f1[None, :, :], (WS, WS, dim)),
    ], -1).reshape(NTOK, 2 * dim)
    return np.cos(f), np.sin(f)   # [196, 64] fp32


def _emit(nc, tc, ctx, aps, has_b, nwin_total=NWIN, loop_n=1):
    pairs = []
    w = 0
    while w < nwin_total:
        pairs.append((w, w + 1) if w + 1 < nwin_total else (w,))
        w += 2

    # x DRAM view: [768, toks, 2] bf16 (fp32 reinterpreted); [..., 1] is
    # the high half = truncated bf16.
    if X_LOAD == "trunc":
        xTb = aps["xT"][:, :, 1:2].rearrange("(k p) n one -> p k (n one)", p=P)
    else:
        xT32 = aps["xT"].rearrange("(k p) n -> p k n", p=P)
    yT = aps["yT"].rearrange("(k p) n -> p k n", p=P)
    w3d = aps["w3"].rearrange("(k p) m -> p k m", p=P)    # [128, 16, 768]

    # Pin the activation table to natural_log_exp_and_others (id 6): it
    # covers every ACT func used (Exp/Ln/Copy/Identity/Square), so the
    # table-load pass inserts no further loads. Without this, Exp maps to
    # set 0 and Ln to set 5 and the pass thrashes (~2.7us per switch).
    nc.scalar.add_instruction(mybir.InstLoadActFuncSet(
        name=nc.get_next_instruction_name(), act_func_set_id=6, ins=[], outs=[]))

    PB = dict(POOL_BUFS)
    consts = ctx.enter_context(tc.tile_pool(name="consts", bufs=1))
    wpool = ctx.enter_context(tc.tile_pool(name="weights", bufs=1))
    w3pool = ctx.enter_context(tc.tile_pool(name="w3s", bufs=2))
    xpool = ctx.enter_context(tc.tile_pool(name="x", bufs=2))
    sqpool = ctx.enter_context(tc.tile_pool(name="sq", bufs=PB["sq"]))
    rowpool = ctx.enter_context(tc.tile_pool(name="rows", bufs=PB["rows"]))
    bcpool = ctx.enter_context(tc.tile_pool(name="bc", bufs=2))
    hpool = ctx.enter_context(tc.tile_pool(name="h", bufs=PB["h"]))
    tpool = ctx.enter_context(tc.tile_pool(name="tmp", bufs=2))
    qkpool = ctx.enter_context(tc.tile_pool(name="qk", bufs=PB["qk"]))
    ropepool = ctx.enter_context(tc.tile_pool(name="rope", bufs=PB["rope"]))
    vpool = ctx.enter_context(tc.tile_pool(name="v", bufs=2))
    epool = ctx.enter_context(tc.tile_pool(name="exp", bufs=PB["exp"]))
    zpool = ctx.enter_context(tc.tile_pool(name="z", bufs=PB["z"]))
    opool = ctx.enter_context(tc.tile_pool(name="ohat", bufs=PB["ohat"]))
    x1pool = ctx.enter_context(tc.tile_pool(name="x1", bufs=2))
    mlppool = ctx.enter_context(tc.tile_pool(name="mlp", bufs=PB["mlp"]))
    gpool = ctx.enter_context(tc.tile_pool(name="g", bufs=PB["g"]))
    ypool = ctx.enter_context(tc.tile_pool(name="y", bufs=2))

    ps_mm = ctx.enter_context(tc.tile_pool(name="psmm", bufs=PB["psmm"], space="PSUM"))
    ps_att = ctx.enter_context(tc.tile_pool(name="psatt", bufs=PB["psatt"], space="PSUM"))
    ps_rot = ctx.enter_context(tc.tile_pool(name="psrot", bufs=PB["psrot"], space="PSUM"))
    ps_stat = ctx.enter_context(tc.tile_pool(name="psstat", bufs=1, space="PSUM"))

    # --- constants / weights in SBUF ---
    def load_w(name, kdim, mdim):
        t = wpool.tile([P, kdim // P, mdim], BF16, tag=name)
        nc.sync.dma_start(t[:], aps[name].rearrange("(k p) m -> p k m", p=P))
        return t

    wq = load_w("wq", DIM, DIM)
    wk = load_w("wk", DIM, DIM)
    wv = load_w("wv", DIM, DIM)
    wp = load_w("wp", DIM, DIM)
    w1 = load_w("w1", DIM, HID)
    w2 = load_w("w2", DIM, HID)

    cos2 = consts.tile([P, PC2], BF16, tag="cos2")
    nc.sync.dma_start(cos2[:], aps["cos2"][:])
    sin2 = consts.tile([P, PC2], BF16, tag="sin2")
    nc.sync.dma_start(sin2[:], aps["sin2"][:])
    r2t = consts.tile([P, P], BF16, tag="r2t")
    nc.sync.dma_start(r2t[:], aps["r2t"][:])
    w3c = consts.tile([1, DIM], BF16, tag="w3c")
    nc.sync.dma_start(w3c[:], aps["w3c"][:])
    colones = consts.tile([P, 1], BF16, tag="colones")
    nc.vector.memset(colones[:], 1.0)
    rowones = consts.tile([1, P], BF16, tag="rowones")
    nc.vector.memset(rowones[:], 1.0)
    epsc = consts.tile([1, 1], F32, tag="epsc")
    nc.vector.memset(epsc[:], EPS)

    def bias_col(name, feat):
        if aps.get(name) is None:
            return None
        t = consts.tile([P, feat // P], F32, tag=name)
        nc.sync.dma_start(t[:], aps[name].rearrange("(k p) -> p k", p=P))
        return t

    qb = bias_col("qb", DIM)
    kb = bias_col("kb", DIM)
    vb = bias_col("vb", DIM)
    pb = bias_col("pb", DIM)
    w1b = bias_col("w1b", HID)
    w2b = bias_col("w2b", HID)
    w3b = bias_col("w3b", DIM)
    vbr = None
    if has_b.get("vb"):
        vbr = consts.tile([1, DIM], BF16, tag="vbr")
        nc.sync.dma_start(vbr[:], aps["vbr"][:])

    CHUNKS = [(0, P), (P, NTOK - P)]   # [128, 68] token chunks per window

    def emit_all_pairs():
        for wins in pairs:
            nwin = len(wins)
            pc = NTOK * nwin
            c0 = wins[0] * NTOK

            xb = xpool.tile([P, KT, PC2], BF16, tag="xb")
            if X_LOAD == "trunc":
                for k in range(KT):
                    nc.sync.dma_start(xb[:, k, :pc], xTb[:, k, c0:c0 + pc])
            else:
                for k in range(KT):
                    x32 = tpool.tile([P, PC2], F32, tag="x32k")
                    nc.sync.dma_start(x32[:, :pc], xT32[:, k, c0:c0 + pc])
                    nc.scalar.activation(out=xb[:, k, :pc], in_=x32[:, :pc],
                                         func=AF.Copy, bias=0.0, scale=1.0)

            # ---------- LN stats: sums via PE, tail via ScalarE+DVE ----
            def ln_stats(src, kt, inv_n):
                # all s1 matmuls first: the s2 ones then never stall the PE
                # FIFO waiting on the DVE squares
                s1 = ps_stat.tile([1, PC2], F32, tag="s1")
                s2 = ps_stat.tile([1, PC2], F32, tag="s2")
                xsqs = []
                for k in range(kt):
                    xsq = sqpool.tile([P, PC2], BF16, tag=f"xsq{k % 3}")
                    nc.vector.tensor_tensor(out=xsq[:, :pc], in0=src[:, k, :pc],
                                            in1=src[:, k, :pc], op=OP.mult)
                    xsqs.append(xsq)
                    nc.tensor.matmul(s1[:, :pc], lhsT=colones[:, 0:1],
                                     rhs=src[:, k, :pc], start=(k == 0),
                                     stop=(k == kt - 1), skip_group_check=True)
                for k in range(kt):
                    nc.tensor.matmul(s2[:, :pc], lhsT=colones[:, 0:1],
                                     rhs=xsqs[k][:, :pc], start=(k == 0),
                                     stop=(k == kt - 1), skip_group_check=True)
                return s1, s2

            def ln_tail(s1, s2, inv_n, want_bcast_mu=True):
                # mu = s1/n (bf16 row); var = s2/n - mu^2 (fp32 exact scale)
                mu_row = rowpool.tile([1, PC2], BF16, tag="mur")
                nc.scalar.activation(out=mu_row[:, :pc], in_=s1[:, :pc],
                                     func=AF.Identity, bias=0.0, scale=inv_n)
                msq = rowpool.tile([1, PC2], F32, tag="msq")
                nc.scalar.activation(out=msq[:, :pc], in_=s1[:, :pc],
                                     func=AF.Square, bias=0.0, scale=inv_n)
                varr = rowpool.tile([1, PC2], F32, tag="varr")
                nc.vector.scalar_tensor_tensor(out=varr[:, :pc], in0=s2[:, :pc],
                                               scalar=inv_n, in1=msq[:, :pc],
                                               op0=OP.mult, op1=OP.subtract)
                lnv = rowpool.tile([1, PC2], F32, tag="lnv")
                nc.scalar.activation(out=lnv[:, :pc], in_=varr[:, :pc],
                                     func=AF.Ln, bias=epsc[:], scale=1.0)
                rstd_row = rowpool.tile([1, PC2], BF16, tag="rstdr")
                nc.scalar.activation(out=rstd_row[:, :pc], in_=lnv[:, :pc],
                                     func=AF.Exp, bias=0.0, scale=-0.5)
                rsb_ps = ps_rot.tile([P, PC2], F32, tag="rot")
                nc.tensor.matmul(rsb_ps[:, :pc], lhsT=rowones[:, 0:P],
                                 rhs=rstd_row[:, :pc], start=True, stop=True)
                rstd_b = bcpool.tile([P, PC2], BF16, tag="rstdb")
                nc.scalar.activation(out=rstd_b[:, :pc], in_=rsb_ps[:, :pc],
                                     func=AF.Copy, bias=0.0, scale=1.0)
                mu_b = None
                if want_bcast_mu:
                    mub_ps = ps_rot.tile([P, PC2], F32, tag="rot")
                    nc.tensor.matmul(mub_ps[:, :pc], lhsT=rowones[:, 0:P],
                                     rhs=mu_row[:, :pc], start=True, stop=True)
                    mu_b = bcpool.tile([P, PC2], BF16, tag="mub")
                    nc.scalar.activation(out=mu_b[:, :pc], in_=mub_ps[:, :pc],
                                         func=AF.Copy, bias=0.0, scale=1.0)
                return mu_row, mu_b, rstd_b

            def ln_apply(src, mu_b, rstd_b, tag):
                hh = hpool.tile([P, KT, PC2], BF16, tag=tag)
                for k in range(KT):
                    tmpc = tpool.tile([P, PC2], BF16, tag="tmpc")
                    nc.vector.tensor_tensor(out=tmpc[:, :pc], in0=src[:, k, :pc],
                                            in1=mu_b[:, :pc], op=OP.subtract)
                    nc.vector.tensor_tensor(out=hh[:, k, :pc], in0=tmpc[:, :pc],
                                            in1=rstd_b[:, :pc], op=OP.mult)
                return hh

            s1a, s2a = ln_stats(xb, KT, 1.0 / DIM)
            _, mu1b, rstd1b = ln_tail(s1a, s2a, 1.0 / DIM)
            h1 = ln_apply(xb, mu1b, rstd1b, "h1")

            # ---------- QKV + RoPE (feature-major q/k) ----------
            def emit_qk(wmat, bcol, dest):
                # rope-rot matmul for tile m is emitted after tile m+1's
                # MM group, so the PE FIFO never waits on the qs copy
                def rope_finish(m, qs):
                    rot = ps_rot.tile([P, PC2], F32, tag="rot")
                    nc.tensor.matmul(rot[:, :pc], lhsT=r2t[:], rhs=qs[:, :pc],
                                     start=True, stop=True)
                    t1 = ropepool.tile([P, PC2], BF16, tag="t1")
                    nc.vector.tensor_tensor(out=t1[:, :pc], in0=qs[:, :pc],
                                            in1=cos2[:, :pc], op=OP.mult)
                    t2 = ropepool.tile([P, PC2], BF16, tag="t2")
                    nc.vector.tensor_tensor(out=t2[:, :pc], in0=rot[:, :pc],
                                            in1=sin2[:, :pc], op=OP.mult)
                    nc.vector.tensor_tensor(out=dest[:, m, :pc], in0=t1[:, :pc],
                                            in1=t2[:, :pc], op=OP.add)

                pend = None
                for m in range(KT):
                    ps = ps_mm.tile([P, PC2], F32, tag="mm")
                    for k in range(KT):
                        nc.tensor.matmul(ps[:, :pc], lhsT=wmat[:, k, m * P:(m + 1) * P],
                                         rhs=h1[:, k, :pc], start=(k == 0), stop=(k == KT - 1))
                    qs = ropepool.tile([P, PC2], BF16, tag="qs")
                    if bcol is None:
                        nc.vector.tensor_copy(out=qs[:, :pc], in_=ps[:, :pc])
                    else:
                        nc.scalar.activation(out=qs[:, :pc], in_=ps[:, :pc],
                                             func=AF.Identity, bias=bcol[:, m:m + 1],
                                             scale=1.0)
                    if pend is not None:
                        rope_finish(*pend)
                    pend = (m, qs)
                rope_finish(*pend)

            qhat = qkpool.tile([P, KT, PC2], BF16, tag="qhat")
            khat = qkpool.tile([P, KT, PC2], BF16, tag="khat")
            emit_qk(wq, qb, qhat)
            emit_qk(wk, kb, khat)

            # ---------- V (token-major, per window, with ones column) --
            v_ts = []
            for wi in range(nwin):
                wcol = wi * NTOK
                vt = []
                for ci, (cs, cn) in enumerate(CHUNKS):
                    v_t = vpool.tile([P, HEADS, HD + 1], BF16, tag=f"v{ci}")
                    nc.vector.memset(v_t[:, :, HD:HD + 1], 1.0)
                    for half in range(2):
                        nh = DIM // 2
                        vps = ps_mm.tile([P, PC2], F32, tag="mm")
                        for k in range(KT):
                            nc.tensor.matmul(vps[0:cn, 0:nh],
                                             lhsT=h1[:, k, wcol + cs:wcol + cs + cn],
                                             rhs=wv[:, k, half * nh:(half + 1) * nh],
                                             start=(k == 0), stop=False if vbr is not None else (k == KT - 1),
                                             skip_group_check=True)
                        if vbr is not None:
                            nc.tensor.matmul(vps[0:cn, 0:nh],
                                             lhsT=rowones[:, 0:cn],
                                             rhs=vbr[:, half * nh:(half + 1) * nh],
                                             start=False, stop=True, skip_group_check=True)
                        nc.scalar.activation(
                            out=v_t[0:cn, half * (HEADS // 2):(half + 1) * (HEADS // 2), 0:HD],
                            in_=vps[0:cn, 0:nh].rearrange("p (h d) -> p h d", d=HD),
                            func=AF.Copy, bias=0.0, scale=1.0)
                    vt.append(v_t)
                v_ts.append(vt)

            # ---------- attention per window/head ----------
            ohat = opool.tile([P, KT, PC2], BF16, tag="ohat")
            for wi in range(nwin):
                wcol = wi * NTOK

                def head_tail(hh, es):
                    r0 = 64 * (hh % 2)
                    g6 = hh // 2
                    ops = ps_att.tile([P, PC2], F32, tag="att")
                    for ci, (cs, cn) in enumerate(CHUNKS):
                        nc.tensor.matmul(ops[0:HD + 1, 0:NTOK], lhsT=v_ts[wi][ci][0:cn, hh, :],
                                         rhs=es[ci][0:cn, :], start=(ci == 0), stop=(ci == 1))
                    zrow = zpool.tile([1, NTOK], BF16, tag="zrow")
                    if RECIP_MODE == "lnexp":
                        zl = zpool.tile([1, NTOK], F32, tag="zl")
                        nc.scalar.activation(out=zl[:], in_=ops[HD:HD + 1, 0:NTOK],
                                             func=AF.Ln, bias=0.0, scale=1.0)
                        nc.scalar.activation(out=zrow[:], in_=zl[:],
                                             func=AF.Exp, bias=0.0, scale=-1.0)
                    else:
                        with nc.allow_low_precision(reason="softmax denom bf16"):
                            nc.vector.reciprocal(out=zrow[:], in_=ops[HD:HD + 1, 0:NTOK])
                    # broadcast 1/z on GPSIMD: off the PE's in-order queue
                    zb = zpool.tile([64, NTOK], BF16, tag="zb")
                    nc.gpsimd.partition_broadcast(zb[:], zrow[:], channels=64)
                    o_sb = zpool.tile([64, NTOK], BF16, tag="osb")
                    nc.vector.tensor_copy(out=o_sb[:], in_=ops[0:64, 0:NTOK])
                    osl = ohat[r0:r0 + 64, g6, wcol:wcol + NTOK]
                    nc.vector.tensor_tensor(out=osl, in0=o_sb[:], in1=zb[:],
                                            op=OP.mult)

                # software-pipelined: head h's PV/softmax tail is emitted
                # after head h+1's QK+exp, so PV never stalls the PE FIFO
                pend = None
                for hh in range(HEADS):
                    r0 = 64 * (hh % 2)
                    g6 = hh // 2
                    qsl = qhat[r0:r0 + 64, g6, wcol:wcol + NTOK]
                    ksl = khat[r0:r0 + 64, g6, wcol:wcol + NTOK]
                    es = []
                    for ci, (cs, cn) in enumerate(CHUNKS):
                        sps = ps_att.tile([P, PC2], F32, tag="att")
                        nc.tensor.matmul(sps[0:cn, 0:NTOK], lhsT=ksl[:, cs:cs + cn],
                                         rhs=qsl, start=True, stop=True)
                        e = epool.tile([P, NTOK], BF16, tag=f"e{ci}")
                        nc.scalar.activation(out=e[0:cn, :], in_=sps[0:cn, 0:NTOK],
                                             func=AF.Exp, bias=0.0, scale=1.0)
                        es.append(e)
                    if pend is not None:
                        head_tail(*pend)
                    pend = (hh, es)
                head_tail(*pend)

            # ---------- proj + residual ----------
            x1 = x1pool.tile([P, KT, PC2], BF16, tag="x1")
            for m in range(KT):
                pps = ps_mm.tile([P, PC2], F32, tag="mm")
                for k in range(KT):
                    nc.tensor.matmul(pps[:, :pc], lhsT=wp[:, k, m * P:(m + 1) * P],
                                     rhs=ohat[:, k, :pc], start=(k == 0), stop=(k == KT - 1))
                if pb is None:
                    nc.vector.tensor_tensor(out=x1[:, m, :pc], in0=pps[:, :pc],
                                            in1=xb[:, m, :pc], op=OP.add)
                else:
                    nc.vector.scalar_tensor_tensor(out=x1[:, m, :pc], in0=pps[:, :pc],
                                                   scalar=pb[:, m:m + 1],
                                                   in1=xb[:, m, :pc],
                                                   op0=OP.add, op1=OP.add)

            # ---------- LN2 + MLP ----------
            s1b, s2b = ln_stats(x1, KT, 1.0 / DIM)
            _, mu2b, rstd2b = ln_tail(s1b, s2b, 1.0 / DIM)
            h2 = ln_apply(x1, mu2b, rstd2b, "h2")

            g = gpool.tile([P, MT, PC2], BF16, tag="g")
            sg = ps_stat.tile([1, PC2], F32, tag="s1")
            ssg = ps_stat.tile([1, PC2], F32, tag="s2")

            def hid_stats(pm, pgsq, stop):
                nc.tensor.matmul(sg[:, :pc], lhsT=colones[:, 0:1], rhs=g[:, pm, :pc],
                                 start=(pm == 0), stop=stop, skip_group_check=True)
                nc.tensor.matmul(ssg[:, :pc], lhsT=colones[:, 0:1], rhs=pgsq[:, :pc],
                                 start=(pm == 0), stop=stop, skip_group_check=True)

            pendg = None
            for m in range(MT):
                p1 = ps_mm.tile([P, PC2], F32, tag="mm")
                for k in range(KT):
                    nc.tensor.matmul(p1[:, :pc], lhsT=w1[:, k, m * P:(m + 1) * P],
                                     rhs=h2[:, k, :pc], start=(k == 0), stop=(k == KT - 1))
                sf = mlppool.tile([P, PC2], BF16, tag="sf")
                if SILU_MODE == "silu":
                    nc.scalar.activation(out=sf[:, :pc], in_=p1[:, :pc], func=AF.Silu,
                                         bias=w1b[:, m:m + 1] if w1b is not None else 0.0,
                                         scale=1.0)
                elif SILU_MODE == "expln" and w1b is None:
                    # sigma(p1) = exp(-ln(1 + exp(-p1))); all funcs in the
                    # pinned table set.
                    e1 = mlppool.tile([P, PC2], BF16, tag="e1")
                    nc.scalar.activation(out=e1[:, :pc], in_=p1[:, :pc], func=AF.Exp,
                                         bias=0.0, scale=-1.0)
                    dd = mlppool.tile([P, PC2], BF16, tag="dd")
                    nc.vector.tensor_scalar_add(out=dd[:, :pc], in0=e1[:, :pc],
                                                scalar1=1.0)
                    ll = mlppool.tile([P, PC2], F32, tag="ll")
                    nc.scalar.activation(out=ll[:, :pc], in_=dd[:, :pc], func=AF.Ln,
                                         bias=0.0, scale=1.0)
                    ss = mlppool.tile([P, PC2], BF16, tag="ss")
                    nc.scalar.activation(out=ss[:, :pc], in_=ll[:, :pc], func=AF.Exp,
                                         bias=0.0, scale=-1.0)
                    if w1b is None:
                        nc.vector.tensor_tensor(out=sf[:, :pc], in0=p1[:, :pc],
                                                in1=ss[:, :pc], op=OP.mult)
                    else:
                        nc.vector.scalar_tensor_tensor(
                            out=sf[:, :pc], in0=p1[:, :pc],
                            scalar=w1b[:, m:m + 1],
                            in1=ss[:, :pc], op0=OP.add, op1=OP.mult)
                else:
                    s1t = mlppool.tile([P, PC2], BF16, tag="s1t")
                    nc.scalar.activation(out=s1t[:, :pc], in_=p1[:, :pc], func=AF.Sigmoid,
                                         bias=w1b[:, m:m + 1] if w1b is not None else 0.0,
                                         scale=1.0)
                    nc.vector.scalar_tensor_tensor(
                        out=sf[:, :pc], in0=p1[:, :pc],
                        scalar=w1b[:, m:m + 1] if w1b is not None else 0.0,
                        in1=s1t[:, :pc], op0=OP.add, op1=OP.mult)
                p2 = ps_mm.tile([P, PC2], F32, tag="mm")
                for k in range(KT):
                    nc.tensor.matmul(p2[:, :pc], lhsT=w2[:, k, m * P:(m + 1) * P],
                                     rhs=h2[:, k, :pc], start=(k == 0), stop=(k == KT - 1))
                if w2b is None:
                    nc.vector.tensor_tensor(out=g[:, m, :pc], in0=p2[:, :pc],
                                            in1=sf[:, :pc], op=OP.mult)
                else:
                    nc.vector.scalar_tensor_tensor(out=g[:, m, :pc], in0=p2[:, :pc],
                                                   scalar=w2b[:, m:m + 1],
                                                   in1=sf[:, :pc],
                                                   op0=OP.add, op1=OP.mult)
                gsq = sqpool.tile([P, PC2], BF16, tag="gsq")
                nc.vector.tensor_tensor(out=gsq[:, :pc], in0=g[:, m, :pc],
                                        in1=g[:, m, :pc], op=OP.mult)
                # stat matmuls lag one m-tile so they never stall the PE
                if pendg is not None:
                    hid_stats(pendg[0], pendg[1], stop=False)
                pendg = (m, gsq)
            hid_stats(pendg[0], pendg[1], stop=True)

            mu3r, _, rstd3b = ln_tail(sg, ssg, 1.0 / HID, want_bcast_mu=False)

            # ---------- w3 (streamed) + ln3-scale + residual -> out ----
            for m in range(KT):
                w3t = w3pool.tile([P, MT, P], BF16, tag="w3t")
                nc.sync.dma_start(w3t[:], w3d[:, :, m * P:(m + 1) * P])
                wps = ps_mm.tile([P, PC2], F32, tag="mm")
                for k in range(MT):
                    nc.tensor.matmul(wps[:, :pc], lhsT=w3t[:, k, :],
                                     rhs=g[:, k, :pc], start=(k == 0), stop=False,
                                     skip_group_check=True)
                # mean-centering of g folded in as a K=1 correction row
                nc.tensor.matmul(wps[:, :pc], lhsT=w3c[:, m * P:(m + 1) * P],
                                 rhs=mu3r[:, :pc], start=False, stop=True,
                                 skip_group_check=True)
                yt = ypool.tile([P, PC2], F32, tag="yt")
                nc.vector.tensor_tensor(out=yt[:, :pc], in0=wps[:, :pc],
                                        in1=rstd3b[:, :pc], op=OP.mult)
                if w3b is None:
                    nc.vector.tensor_tensor(out=yt[:, :pc], in0=yt[:, :pc],
                                            in1=x1[:, m, :pc], op=OP.add)
                else:
                    nc.vector.scalar_tensor_tensor(out=yt[:, :pc], in0=yt[:, :pc],
                                                   scalar=w3b[:, m:m + 1],
                                                   in1=x1[:, m, :pc],
                                                   op0=OP.add, op1=OP.add)
                nc.sync.dma_start(yT[:, m, c0:c0 + pc], yt[:, :pc])

    if loop_n > 1:
        with tc.For_i(0, loop_n, 1):
            emit_all_pairs()
    else:
        emit_all_pairs()


def _build(has_biases, nwin_total=NWIN, ncores=N_CORES, loop_n=1):
    key = ("prog", tuple(sorted(has_biases.items())), nwin_total, ncores, loop_n,
           SILU_MODE, RECIP_MODE, X_LOAD, tuple(sorted(POOL_BUFS.items())))
    if key in _cache:
        return _cache[key]
    nc = bacc.Bacc("TRN2", target_bir_lowering=False, debug=False,
                   enable_asserts=False, num_devices=ncores)
    toks = nwin_total * NTOK
    aps = {}
    if X_LOAD == "trunc":
        aps["xT"] = nc.dram_tensor("xT", [DIM, toks, 2], BF16, kind="ExternalInput").ap()
    else:
        aps["xT"] = nc.dram_tensor("xT", [DIM, toks], F32, kind="ExternalInput").ap()
    aps["yT"] = nc.dram_tensor("yT", [DIM, toks], F32, kind="ExternalOutput").ap()
    for nm, shp in [("wq", [DIM, DIM]), ("wk", [DIM, DIM]), ("wv", [DIM, DIM]),
                    ("wp", [DIM, DIM]), ("w1", [DIM, HID]), ("w2", [DIM, HID]),
                    ("w3", [HID, DIM])]:
        aps[nm] = nc.dram_tensor(nm, shp, BF16, kind="ExternalInput").ap()
    aps["cos2"] = nc.dram_tensor("cos2", [P, PC2], BF16, kind="ExternalInput").ap()
    aps["sin2"] = nc.dram_tensor("sin2", [P, PC2], BF16, kind="ExternalInput").ap()
    aps["r2t"] = nc.dram_tensor("r2t", [P, P], BF16, kind="ExternalInput").ap()
    aps["w3c"] = nc.dram_tensor("w3c", [1, DIM], BF16, kind="ExternalInput").ap()
    bias_specs = {"qb": DIM, "kb": DIM, "vb": DIM, "pb": DIM,
                  "w1b": HID, "w2b": HID, "w3b": DIM}
    for nm, d in bias_specs.items():
        if has_biases.get(nm):
            aps[nm] = nc.dram_tensor(nm, [d], F32, kind="ExternalInput").ap()
        else:
            aps[nm] = None
    if has_biases.get("vb"):
        aps["vbr"] = nc.dram_tensor("vbr", [1, DIM], BF16, kind="ExternalInput").ap()
    with tile.TileContext(nc) as tc:
        with ExitStack() as ctx:
            _emit(nc, tc, ctx, aps, has_biases, nwin_total, loop_n)
    nc.compile()
    _cache[key] = nc
    return nc


def _host_prep(inputs):
    f = {k: np.asarray(v, np.float32) if hasattr(v, "shape") else v
         for k, v in inputs.items()}
    scale = HD ** -0.5
    wq = f["ln1_w"][:, None] * f["q_w"] * scale
    wk = f["ln1_w"][:, None] * f["k_w"]
    wv = f["ln1_w"][:, None] * f["v_w"]
    qb = (f["ln1_b"] @ f["q_w"] + f["q_b"]) * scale
    kb = f["ln1_b"] @ f["k_w"]
    vb = f["ln1_b"] @ f["v_w"] + f["v_b"]
    wp = f["proj_w"]
    pb = f["proj_b"]
    w1 = f["ln2_w"][:, None] * f["w1_w"]
    w2 = f["ln2_w"][:, None] * f["w2_w"]
    w1b = f["ln2_b"] @ f["w1_w"] + f["w1_b"]
    w2b = f["ln2_b"] @ f["w2_w"] + f["w2_b"]
    w3 = f["ffn_w"][:, None] * f["w3_w"]
    w3b = f["ffn_b"] @ f["w3_w"] + f["w3_b"]
    w3c = -w3.sum(0).reshape(1, DIM)   # -colsum for mean-centering row

    cos, sin = _rope_tables()
    cosT = np.ascontiguousarray(cos.T)
    sinT = np.ascontiguousarray(sin.T)
    cos2 = np.tile(np.concatenate([cosT, cosT], 0), (1, 2))   # [128, 392]
    sin2 = np.tile(np.concatenate([sinT, sinT], 0), (1, 2))

    r = np.zeros((64, 64), np.float32)
    for i in range(32):
        r[2 * i, 2 * i + 1] = -1.0
        r[2 * i + 1, 2 * i] = 1.0
    r2 = np.zeros((128, 128), np.float32)
    r2[:64, :64] = r
    r2[64:, 64:] = r
    r2t = np.ascontiguousarray(r2.T)

    x = f["x"]
    pad = (-H) % WS
    nw = (H + pad) // WS
    xp = np.pad(x, ((0, 0), (0, pad), (0, pad), (0, 0)))
    t = xp.reshape(B, nw, WS, nw, WS, DIM).transpose(0, 1, 3, 2, 4, 5).reshape(B, NWIN * NTOK, DIM)

    shared = {
        "wq": wq.astype(BF16NP), "wk": wk.astype(BF16NP), "wv": wv.astype(BF16NP),
        "wp": wp.astype(BF16NP), "w1": w1.astype(BF16NP), "w2": w2.astype(BF16NP),
        "w3": w3.astype(BF16NP), "w3c": w3c.astype(BF16NP),
        "cos2": cos2.astype(BF16NP), "sin2": sin2.astype(BF16NP),
        "r2t": r2t.astype(BF16NP),
    }
    biases = {"qb": qb, "kb": kb, "vb": vb, "pb": pb, "w1b": w1b, "w2b": w2b, "w3b": w3b}
    has_biases = {k: bool(np.any(v != 0.0)) for k, v in biases.items()}
    for k, v in biases.items():
        if has_biases[k]:
            shared[k] = np.ascontiguousarray(v, np.float32)
    if has_biases["vb"]:
        shared["vbr"] = vb.reshape(1, DIM).astype(BF16NP)

    in_maps = []
    for b in range(B):
        m = dict(shared)
        xt32 = np.ascontiguousarray(t[b].T)     # [768, 4900] fp32
        if X_LOAD == "trunc":
            m["xT"] = xt32.view(BF16NP).reshape(DIM, TOKS, 2)
        else:
            m["xT"] = xt32
        in_maps.append(m)
    return in_maps, has_biases


def _host_post(results):
    pad = (-H) % WS
    nw = (H + pad) // WS
    Hp = H + pad
    y = np.empty((B, H, W, DIM), np.float32)
    for b in range(B):
        yb = np.asarray(results[b]["yT"])
        yw = yb.T.reshape(nw, nw, WS, WS, DIM).transpose(0, 2, 1, 3, 4).reshape(Hp, Hp, DIM)
        y[b] = yw[:H, :W, :]
    return y


def kernel(**inputs):
    in_maps, has_biases = _host_prep(inputs)
    nc = _build(has_biases)
    res = run_bass_kernel_spmd(nc, in_maps, core_ids=list(range(N_CORES)))
    return _host_post(res.results)
